# revision 1
# baseline (speedup 1.0000x reference)
"""Trainium2 Bass kernel for nn_NodeModel (GNN message passing).

reference:
    agg = segment_sum(edge_attr, edge_index[0], num_segments=100000)   # [N, 64]
    h = concat([x, agg, u[v_indices]], axis=1)                         # [N, 256]
    out = relu(h @ W1 + b1) @ W2 + b2                                  # [N, 128]

Strategy (8 NeuronCores, SPMD, no collectives):
  - Shard nodes across cores (12500/core); shard edges by destination-node
    partition (host buckets+sorts edges by the core/block owning their row).
  - Within a core, nodes are processed in blocks of 128. Edges are sorted by
    row, grouped per block, padded to T tiles of 128 edges.
  - segment_sum on device: per 128-edge tile, build a one-hot matrix
    P[e, m] = (row_local[e] == m) with DVE/GPSIMD tensor_scalar(is_equal),
    then TensorE matmul:  aggT_hilo += ea_hilo.T @ P  accumulated in PSUM.
  - edge_attr is split hi/lo bf16 (exact to ~1e-5, same total bytes as fp32).
    The hi and lo partial sums land on PSUM partitions 0-63 / 64-127; they
    are merged for free inside the MLP by duplicating W1's agg-rows.
  - MLP runs feature-major (transposed) with fp32r matmuls, N=512 node
    groups. x and u[v_indices] are pre-transposed on host; the output is
    produced transposed and un-transposed on host.
"""

import sys

sys.path.insert(0, "/opt/trn_rl_repo")

import numpy as np
import ml_dtypes

import concourse.bass as bass
import concourse.mybir as mybir
from concourse import bacc, tile
from concourse.bass_utils import run_bass_kernel_spmd

bf16 = ml_dtypes.bfloat16

D_X, D_E, D_U = 128, 64, 64
D_HID, D_OUT = 256, 128
NB = 128  # nodes per block
WIN = 4   # 32-node one-hot windows per block

FULL_CFG = dict(n_cores=8, n_nodes=100000, npc=12500, blocks=98, group=4)

_cache = {}


def _build_nc(Tb, blocks, npad, group, n_cores=8, reps=1, opts=None):
    """Build the SPMD Bass program. Tb = per-block edge tile counts.

    reps > 1 wraps the whole computation in a hardware For_i loop — used
    only for timing (per-iteration time = delta(wall)/delta(reps), which
    cancels the host dispatch overhead)."""
    opts = dict(opts or {})
    skip_mlp = opts.get("skip_mlp", False)
    skip_edges = opts.get("skip_edges", False)
    p_bufs = opts.get("p_bufs", 20)
    ea_bufs = opts.get("ea_bufs", 5)
    gp_mod = opts.get("gp_mod", 999)
    act_mod = opts.get("act_mod", 8)  # t % act_mod == 5 -> ACT pair
    xt_ring = opts.get("xt_ring", "sync")
    Tb = list(Tb)
    offs = [0]
    for t in Tb:
        offs.append(offs[-1] + t)
    TT = offs[-1]
    max_blk_tiles = max(
        sum(Tb[b * WIN : (b + 1) * WIN]) for b in range(blocks)
    )
    nc = bacc.Bacc(
        "TRN2", target_bir_lowering=False, debug=False, num_devices=n_cores
    )
    f32, rf32, b16 = mybir.dt.float32, mybir.dt.float32r, mybir.dt.bfloat16

    # partition-major: partition = edge slot within tile, free = (block, tile, m)
    ea_in = nc.declare_dram_parameter("ea", [128, TT * 128], b16, isOutput=False)
    # idx carries a bf16 iota [128,128] packed into its first 64 f32 columns
    idx_in = nc.declare_dram_parameter("idx", [128, 64 + TT], f32, isOutput=False)
    xT_in = nc.declare_dram_parameter("xT", [128, npad], rf32, isOutput=False)
    ugT_in = nc.declare_dram_parameter("ugT", [64, npad], rf32, isOutput=False)
    # weight layouts are partition-major: [K-part, mh, M]
    w1x_in = nc.declare_dram_parameter("w1x", [128, 2, 128], rf32, isOutput=False)
    w1a_in = nc.declare_dram_parameter("w1a", [128, 2, 128], rf32, isOutput=False)
    w1u_in = nc.declare_dram_parameter("w1u", [64, 2, 128], rf32, isOutput=False)
    w2_in = nc.declare_dram_parameter("w2", [128, 2, 128], rf32, isOutput=False)
    b1_in = nc.declare_dram_parameter("b1", [128, 2], f32, isOutput=False)
    b2_in = nc.declare_dram_parameter("b2", [128, 1], f32, isOutput=False)
    outT = nc.declare_dram_parameter("outT", [128, npad], f32, isOutput=True)

    n_groups = (blocks + group - 1) // group

    with tile.TileContext(nc) as tc:
        with (
            tc.tile_pool(name="const", bufs=1) as cpool,
            tc.tile_pool(name="xt", bufs=1) as xpool,
            tc.tile_pool(name="ea", bufs=ea_bufs) as eapool,
            tc.tile_pool(name="p", bufs=p_bufs) as ppool,
            tc.tile_pool(name="hag", bufs=3) as hagpool,
            tc.tile_pool(name="ug", bufs=2) as ugpool,
            tc.tile_pool(name="h1", bufs=4) as h1pool,
            tc.tile_pool(name="outs", bufs=2) as opool,
            tc.tile_pool(name="ps_agg", bufs=4, space="PSUM") as agg_ps_pool,
            tc.tile_pool(name="ps_o1", bufs=2, space="PSUM") as o1_ps_pool,
            tc.tile_pool(name="ps_o2", bufs=2, space="PSUM") as o2_ps_pool,
        ):
          def _emit_body():
              # ---- constants / resident tensors ----
              idx_t = cpool.tile([128, 64 + TT], f32, tag="idx")
              nc.sync.dma_start(idx_t[:], idx_in[:])
              iota_ap = idx_t[:, 0:64].bitcast(b16)  # [128, 128] bf16 iota
              w1x_t = cpool.tile([128, 2, 128], rf32, tag="w1x")
              nc.sync.dma_start(w1x_t[:], w1x_in[:])
              w1a_t = cpool.tile([128, 2, 128], rf32, tag="w1a")
              nc.sync.dma_start(w1a_t[:], w1a_in[:])
              w1u_t = cpool.tile([64, 2, 128], rf32, tag="w1u")
              nc.sync.dma_start(w1u_t[:], w1u_in[:])
              w2_t = cpool.tile([128, 2, 128], rf32, tag="w2")
              nc.sync.dma_start(w2_t[:], w2_in[:])
              b1_t = cpool.tile([128, 2], f32, tag="b1")
              nc.sync.dma_start(b1_t[:], b1_in[:])
              b2_t = cpool.tile([128, 1], f32, tag="b2")
              nc.sync.dma_start(b2_t[:], b2_in[:])

              xT_t = xpool.tile([128, npad], rf32, tag="xT")
              # load x in chunks so early groups can start sooner
              xchunk = 8 * NB
              xt_eng = nc.scalar if xt_ring == "scalar" else nc.sync
              for s in range(0, npad, xchunk):
                  e = min(s + xchunk, npad)
                  xt_eng.dma_start(xT_t[:, s:e], xT_in[:, s:e])

              hag_tiles = {}
              # ---- edge scatter-add per block ----
              for b in range(blocks if not skip_edges else 0):
                  g, bi = divmod(b, group)
                  if bi == 0:
                      gw = min(group, blocks - g * group) * NB
                      hag_tiles[g] = hagpool.tile(
                          [128, group * NB], rf32, tag="hag", name=f"hag{g}"
                      )
                  Tws = Tb[b * WIN : (b + 1) * WIN]
                  o_b = offs[b * WIN]
                  Tblk = sum(Tws)
                  ea_t = eapool.tile(
                      [128, max_blk_tiles * 128], b16, tag="ea", name=f"ea{b}"
                  )
                  nc.sync.dma_start(
                      ea_t[:, : Tblk * 128],
                      ea_in[:, o_b * 128 : (o_b + Tblk) * 128],
                  )
                  if opts.get("dma_only"):
                      continue
                  agg_ps = agg_ps_pool.tile([128, NB], f32, tag="agg")
                  ti = 0
                  for w in range(WIN):
                      for t in range(Tws[w]):
                          o = o_b + ti
                          p_t = ppool.tile([128, 32], b16, tag="p")
                          idx_col = idx_t[:, 64 + o : 64 + o + 1]
                          if ti % act_mod == 5:
                              # ACT-side one-hot: P = relu(1 - (idx - iota)^2)
                              d2 = ppool.tile(
                                  [128, 32], b16, tag="d2", name=f"d2_{b}_{ti}"
                              )
                              nc.scalar.activation(
                                  out=d2[:], in_=iota_ap[:, 0:32],
                                  func=mybir.ActivationFunctionType.Square,
                                  bias=idx_col, scale=-1.0,
                              )
                              nc.scalar.activation(
                                  out=p_t[:], in_=d2[:],
                                  func=mybir.ActivationFunctionType.Relu,
                                  bias=1.0, scale=-1.0,
                              )
                          else:
                              nc.vector.tensor_scalar(
                                  out=p_t[:],
                                  in0=iota_ap[:, 0:32],
                                  scalar1=idx_col,
                                  scalar2=None,
                                  op0=mybir.AluOpType.is_equal,
                              )
                          nc.tensor.matmul(
                              agg_ps[:, 32 * w : 32 * (w + 1)],
                              ea_t[:, ti * 128 : (ti + 1) * 128],
                              p_t[:],
                              start=(t == 0),
                              stop=(t == Tws[w] - 1),
                          )
                          ti += 1
                  # move [aggT_hi ; aggT_lo] into the MLP's K-chunk staging tile
                  nc.scalar.activation(
                      out=hag_tiles[g][:, bi * NB : (bi + 1) * NB],
                      in_=agg_ps[:],
                      func=mybir.ActivationFunctionType.Copy,
                  )

              # ---- MLP per group of blocks (feature-major) ----
              no_mlp = skip_mlp or opts.get("dma_only")
              for g in range(n_groups if not no_mlp else 0):
                  s = g * group * NB
                  gw = min(group * NB, npad - s)
                  ug_t = ugpool.tile([64, group * NB], rf32, tag="ug")
                  mlp_eng = {"sync": nc.sync, "scalar": nc.scalar}[
                      opts.get("mlp_ring", "scalar")
                  ]
                  mlp_eng.dma_start(ug_t[:, :gw], ugT_in[:, s : s + gw])
                  hag = hag_tiles[g]
                  h1_list = []
                  for mh in range(2):
                      o1 = o1_ps_pool.tile([128, group * NB], f32, tag="o1")
                      nc.tensor.matmul(
                          o1[:, :gw], w1x_t[:, mh, :],
                          xT_t[:, s : s + gw],
                          start=True, stop=False,
                      )
                      nc.tensor.matmul(
                          o1[:, :gw], w1a_t[:, mh, :],
                          hag[:, :gw],
                          start=False, stop=False,
                      )
                      nc.tensor.matmul(
                          o1[:, :gw], w1u_t[:, mh, :],
                          ug_t[:, :gw],
                          start=False, stop=True,
                      )
                      h1 = h1pool.tile([128, group * NB], rf32, tag="h1")
                      nc.scalar.activation(
                          out=h1[:, :gw], in_=o1[:, :gw],
                          func=mybir.ActivationFunctionType.Relu,
                          bias=b1_t[:, mh : mh + 1],
                      )
                      h1_list.append(h1)
                  o2 = o2_ps_pool.tile([128, group * NB], f32, tag="o2")
                  for kh in range(2):
                      nc.tensor.matmul(
                          o2[:, :gw], w2_t[:, kh, :],
                          h1_list[kh][:, :gw],
                          start=(kh == 0), stop=(kh == 1),
                      )
                  out_t = opool.tile([128, group * NB], f32, tag="outs")
                  nc.scalar.activation(
                      out=out_t[:, :gw], in_=o2[:, :gw],
                      func=mybir.ActivationFunctionType.Identity,
                      bias=b2_t[:],
                  )
                  mlp_eng.dma_start(outT[:, s : s + gw], out_t[:, :gw])

          if reps == 1:
              _emit_body()
          else:
              with tc.For_i(0, reps, 1):
                  _emit_body()

    nc.compile()
    return nc


def _pack_inputs(x, edge_index, edge_attr, u, v_indices, W1, b1, W2, b2, cfg):
    """Host-side sharding: bucket + sort edges by destination node partition."""
    n_cores, npc, blocks = cfg["n_cores"], cfg["npc"], cfg["blocks"]
    n_nodes = cfg["n_nodes"]
    npad = blocks * NB
    row = np.asarray(edge_index[0], dtype=np.int64)
    ea = np.ascontiguousarray(np.asarray(edge_attr, dtype=np.float32))
    x = np.asarray(x, dtype=np.float32)
    u = np.asarray(u, dtype=np.float32)
    v_indices = np.asarray(v_indices, dtype=np.int64)
    W1 = np.asarray(W1, dtype=np.float32)
    W2 = np.asarray(W2, dtype=np.float32)
    b1 = np.asarray(b1, dtype=np.float32)
    b2 = np.asarray(b2, dtype=np.float32)
    d_e = ea.shape[1]

    order = np.argsort(row, kind="stable")
    row_s = row[order]
    ea_s = ea[order]
    hi = ea_s.astype(bf16)
    lo = (ea_s - hi.astype(np.float32)).astype(bf16)
    ea_hilo = np.concatenate([hi, lo], axis=1)  # [E, 2*d_e] bf16

    # window boundaries: core c window i covers nodes [npc*c + 32*i, +32),
    # clipped to the core's node range (WIN windows per 128-node block).
    nwin = blocks * WIN
    bases = (npc * np.arange(n_cores)[:, None] + 32 * np.arange(nwin)[None, :]).ravel()
    core_hi = (npc * (1 + np.arange(n_cores))[:, None]).repeat(nwin, 1).ravel()
    starts = np.searchsorted(row_s, np.minimum(bases, core_hi), side="left")
    ends = np.searchsorted(row_s, np.minimum(bases + 32, core_hi), side="left")
    cnts = (ends - starts).reshape(n_cores, nwin)
    Tb = np.maximum(1, (cnts.max(axis=0) + 127) // 128).astype(int)  # [nwin]
    offs = np.concatenate([[0], np.cumsum(Tb)])
    TT = int(offs[-1])

    # ea layout: [core][partition=slot%128, (offs[b] + tile)*128 + m]
    # Vectorized scatter: edge k of block b (rank r within the block) lands in
    # slot offs[b]*128 + r of its core's slot array.
    ea_pack = np.empty((n_cores, 128, TT * 128), dtype=bf16)
    idx_pack = np.empty((n_cores, 128, TT), dtype=np.float32)
    starts2 = starts.reshape(n_cores, nwin)
    for c in range(n_cores):
        cs, ce = starts2[c, 0], ends.reshape(n_cores, nwin)[c, -1]
        wb = np.repeat(np.arange(nwin), cnts[c])            # window id per edge
        rank = np.arange(ce - cs) - np.repeat(starts2[c] - cs, cnts[c])
        slot = offs[wb] * 128 + rank
        coreslots = np.zeros((TT * 128, 2 * d_e), dtype=bf16)
        coreslots[slot] = ea_hilo[cs:ce]
        ea_pack[c] = (
            coreslots.reshape(TT, 128, 2 * d_e)
            .transpose(1, 0, 2)
            .reshape(128, TT * 128)
        )
        ivals = np.zeros(TT * 128, dtype=np.float32)
        ivals[slot] = (row_s[cs:ce] - (npc * c + 32 * wb)).astype(np.float32)
        idx_pack[c] = ivals.reshape(TT, 128).T

    iota = np.broadcast_to(np.arange(128, dtype=np.float32), (128, 128)).astype(bf16)
    iota_f32 = np.ascontiguousarray(iota).view(np.float32)  # [128, 64]
    uT = u.T  # [d_u, n_graphs]

    # weights, partition-major [K, mh, M]
    w1x = np.ascontiguousarray(W1[:D_X].reshape(D_X, 2, 128))
    w1a_single = W1[D_X : D_X + d_e]                       # [64, 256]
    w1a_dup = np.concatenate([w1a_single, w1a_single], 0)  # [128, 256] hi|lo dup
    w1a = np.ascontiguousarray(w1a_dup.reshape(128, 2, 128))
    w1u = np.ascontiguousarray(W1[D_X + d_e :].reshape(D_U, 2, 128))
    w2 = np.ascontiguousarray(W2.reshape(2, 128, D_OUT).transpose(1, 0, 2))
    b1p = np.ascontiguousarray(b1.reshape(2, 128).T)
    b2p = np.ascontiguousarray(b2.reshape(128, 1))

    in_maps = []
    for c in range(n_cores):
        lo_n, hi_n = npc * c, min(npc * (c + 1), n_nodes)
        xT = np.zeros((D_X, npad), dtype=np.float32)
        xT[:, : hi_n - lo_n] = x[lo_n:hi_n].T
        ugT = np.zeros((D_U, npad), dtype=np.float32)
        ugT[:, : hi_n - lo_n] = uT[:, v_indices[lo_n:hi_n]]
        in_maps.append({
            "ea": ea_pack[c],
            "idx": np.concatenate([iota_f32, idx_pack[c]], axis=1),
            "xT": xT,
            "ugT": ugT,
            "w1x": w1x,
            "w1a": w1a,
            "w1u": w1u,
            "w2": w2,
            "b1": b1p,
            "b2": b2p,
        })
    return in_maps, tuple(int(t) for t in Tb)


def _run(inputs, cfg, trace=False, reps=1):
    in_maps, T = _pack_inputs(
        inputs["x"], inputs["edge_index"], inputs["edge_attr"], inputs["u"],
        inputs["v_indices"], inputs["W1"], inputs["b1"], inputs["W2"],
        inputs["b2"], cfg,
    )
    key = (T, cfg["blocks"], cfg["group"], reps)
    if key not in _cache:
        _cache[key] = _build_nc(
            T, cfg["blocks"], cfg["blocks"] * NB, cfg["group"], reps=reps
        )
    nc = _cache[key]
    res = run_bass_kernel_spmd(nc, in_maps, list(range(cfg["n_cores"])), trace=trace)
    n_nodes, npc = cfg["n_nodes"], cfg["npc"]
    out = np.empty((n_nodes, D_OUT), dtype=np.float32)
    for c in range(cfg["n_cores"]):
        lo_n, hi_n = npc * c, min(npc * (c + 1), n_nodes)
        out[lo_n:hi_n] = res.results[c]["outT"].T[: hi_n - lo_n]
    _run.last_results = res
    return out


def kernel(x, edge_index, edge_attr, u, v_indices, W1, b1, W2, b2):
    inputs = dict(x=x, edge_index=edge_index, edge_attr=edge_attr, u=u,
                  v_indices=v_indices, W1=W1, b1=b1, W2=W2, b2=b2)
    return _run(inputs, FULL_CFG)



# revision 3
# speedup vs baseline: 1.9625x; 1.9625x over previous
"""Trainium2 Bass kernel for nn_NodeModel (GNN message passing).

reference:
    agg = segment_sum(edge_attr, edge_index[0], num_segments=100000)   # [N, 64]
    h = concat([x, agg, u[v_indices]], axis=1)                         # [N, 256]
    out = relu(h @ W1 + b1) @ W2 + b2                                  # [N, 128]

Strategy (8 NeuronCores, SPMD, no collectives) — v2:
  - Nodes are PERMUTED on host into 3136 windows of 32 slots (8 cores x 98
    blocks x 4 windows) by a degree-balanced snake deal, so every window owns
    <= 512 edges -> exactly 4 edge tiles of 128 per window, zero cross-core
    padding skew. Output is un-permuted on host.
  - Edges are bucketed by destination window and packed into 128-edge tiles.
  - segment_sum on device: per tile, TensorE matmul aggT += ea_tile.T @ P
    where P[e, m] (one-hot of the edge's within-window position) is built on
    HOST and shipped as fp8 — no on-device one-hot work at all.
  - edge_attr is split hi/lo fp8_e4m3 (hi = fp8(x), lo = fp8(x - hi)); the
    128-column fp8 stationary gets FWL (4x weight-load). hi/lo partial sums
    land on PSUM partitions 0-63 / 64-127 and are merged for free in the MLP
    by duplicating W1's agg-rows.
  - MLP runs feature-major in bf16 (weights, x, u[v_indices], h1 all bf16;
    fp32 PSUM accumulate), N=512 node groups. Output stored bf16, upcast on
    host. End-to-end rel err ~4e-3 (gate 2e-2).
  - Engine split: SP ring = ea DMAs + consts; ACT ring = P/ug DMAs, relu mh0,
    out copy; DVE = hag PSUM->SBUF copies, relu mh1, outT DMAs; PE = matmuls.
"""

import sys

sys.path.insert(0, "/opt/trn_rl_repo")

import numpy as np
import ml_dtypes

import concourse.bass as bass
import concourse.mybir as mybir
from concourse import bacc, tile
from concourse.bass_utils import run_bass_kernel_spmd

bf16 = ml_dtypes.bfloat16
f8 = mybir.dt.np(mybir.dt.float8e4)  # ml_dtypes.float8_e4m3

D_X, D_E, D_U = 128, 64, 64
D_HID, D_OUT = 256, 128
NB = 128   # nodes per block
WIN = 4    # 32-node windows per block
WSZ = 32   # nodes per window

FULL_CFG = dict(n_cores=8, n_nodes=100000, npc=12544, blocks=98, group=4)

_cache = {}


def _build_nc(Tb, blocks, npad, group, n_cores=8, reps=1, opts=None):
    """Build the SPMD Bass program. Tb = per-window edge tile counts
    (blocks*WIN entries, shared across cores).

    reps > 1 wraps the whole computation in a hardware For_i loop — used
    only for timing (per-iteration time = delta(wall)/delta(reps), which
    cancels the host dispatch overhead)."""
    opts = dict(opts or {})
    ea_bufs = opts.get("ea_bufs", 3)
    p_bufs = opts.get("p_bufs", 3)
    nwin = blocks * WIN
    Tb = list(Tb)
    assert len(Tb) == nwin
    offs = [0]
    for t in Tb:
        offs.append(offs[-1] + t)
    TT = offs[-1]
    n_groups = (blocks + group - 1) // group
    # tiles per group of blocks
    gbounds = []
    for g in range(n_groups):
        w0 = g * group * WIN
        w1 = min((g + 1) * group, blocks) * WIN
        gbounds.append((offs[w0], offs[w1]))
    max_gt = max(t1 - t0 for t0, t1 in gbounds)

    nc = bacc.Bacc(
        "TRN2", target_bir_lowering=False, debug=False, num_devices=n_cores
    )
    f32, b16, fp8 = mybir.dt.float32, mybir.dt.bfloat16, mybir.dt.float8e4

    # partition = edge slot within tile; free = (tile, feature)
    ea_in = nc.declare_dram_parameter("ea", [128, TT * 128], fp8, isOutput=False)
    p_in = nc.declare_dram_parameter("p", [128, TT * 32], fp8, isOutput=False)
    xT_in = nc.declare_dram_parameter("xT", [128, npad], b16, isOutput=False)
    ugT_in = nc.declare_dram_parameter("ugT", [64, npad], b16, isOutput=False)
    # weight layouts are partition-major: [K-part, mh, M]
    w1x_in = nc.declare_dram_parameter("w1x", [128, 2, 128], b16, isOutput=False)
    w1a_in = nc.declare_dram_parameter("w1a", [128, 2, 128], b16, isOutput=False)
    w1u_in = nc.declare_dram_parameter("w1u", [64, 2, 128], b16, isOutput=False)
    w2_in = nc.declare_dram_parameter("w2", [128, 2, 128], b16, isOutput=False)
    b1_in = nc.declare_dram_parameter("b1", [128, 2], f32, isOutput=False)
    b2_in = nc.declare_dram_parameter("b2", [128, 1], f32, isOutput=False)
    outT = nc.declare_dram_parameter("outT", [128, npad], b16, isOutput=True)

    with tile.TileContext(nc) as tc:
        with (
            tc.tile_pool(name="const", bufs=1) as cpool,
            tc.tile_pool(name="xt", bufs=1) as xpool,
            tc.tile_pool(name="ea", bufs=ea_bufs) as eapool,
            tc.tile_pool(name="p", bufs=p_bufs) as ppool,
            tc.tile_pool(name="hag", bufs=3) as hagpool,
            tc.tile_pool(name="ug", bufs=2) as ugpool,
            tc.tile_pool(name="h1", bufs=4) as h1pool,
            tc.tile_pool(name="outs", bufs=2) as opool,
            tc.tile_pool(name="ps_agg", bufs=4, space="PSUM") as agg_ps_pool,
            tc.tile_pool(name="ps_o1", bufs=2, space="PSUM") as o1_ps_pool,
            tc.tile_pool(name="ps_o2", bufs=2, space="PSUM") as o2_ps_pool,
        ):
          def _emit_body():
              # ---- constants / resident tensors ----
              w1x_t = cpool.tile([128, 2, 128], b16, tag="w1x")
              nc.sync.dma_start(w1x_t[:], w1x_in[:])
              w1a_t = cpool.tile([128, 2, 128], b16, tag="w1a")
              nc.sync.dma_start(w1a_t[:], w1a_in[:])
              w1u_t = cpool.tile([64, 2, 128], b16, tag="w1u")
              nc.sync.dma_start(w1u_t[:], w1u_in[:])
              w2_t = cpool.tile([128, 2, 128], b16, tag="w2")
              nc.sync.dma_start(w2_t[:], w2_in[:])
              b1_t = cpool.tile([128, 2], f32, tag="b1")
              nc.sync.dma_start(b1_t[:], b1_in[:])
              b2_t = cpool.tile([128, 1], f32, tag="b2")
              nc.sync.dma_start(b2_t[:], b2_in[:])

              xT_t = xpool.tile([128, npad], b16, tag="xT")
              # load x in chunks so early groups can start sooner
              xchunk = 16 * NB
              for s in range(0, npad, xchunk):
                  e = min(s + xchunk, npad)
                  nc.sync.dma_start(xT_t[:, s:e], xT_in[:, s:e])

              for g in range(n_groups):
                  nb = min(group, blocks - g * group)
                  gw = nb * NB
                  t0, t1 = gbounds[g]
                  gt = t1 - t0
                  # ---- edge scatter-add for this group's blocks ----
                  ea_t = eapool.tile([128, max_gt * 128], fp8, tag="ea",
                                     name=f"ea{g}")
                  nc.sync.dma_start(ea_t[:, : gt * 128],
                                    ea_in[:, t0 * 128 : t1 * 128])
                  p_t = ppool.tile([128, max_gt * 32], fp8, tag="p",
                                   name=f"p{g}")
                  nc.scalar.dma_start(p_t[:, : gt * 32],
                                      p_in[:, t0 * 32 : t1 * 32])
                  ug_t = ugpool.tile([64, group * NB], b16, tag="ug")
                  nc.scalar.dma_start(ug_t[:, :gw],
                                      ugT_in[:, g * group * NB :
                                             g * group * NB + gw])
                  hag = hagpool.tile([128, group * NB], b16, tag="hag",
                                     name=f"hag{g}")
                  for bi in range(nb):
                      b = g * group + bi
                      agg_ps = agg_ps_pool.tile([128, NB], f32, tag="agg")
                      for w in range(WIN):
                          wi = b * WIN + w
                          for t in range(Tb[wi]):
                              ti = offs[wi] - t0 + t
                              nc.tensor.matmul(
                                  agg_ps[:, WSZ * w : WSZ * (w + 1)],
                                  ea_t[:, ti * 128 : (ti + 1) * 128],
                                  p_t[:, ti * 32 : (ti + 1) * 32],
                                  start=(t == 0),
                                  stop=(t == Tb[wi] - 1),
                              )
                      # hi|lo partial sums -> MLP staging tile (DVE copy)
                      nc.vector.tensor_scalar(
                          out=hag[:, bi * NB : (bi + 1) * NB],
                          in0=agg_ps[:],
                          scalar1=0.0, scalar2=None,
                          op0=mybir.AluOpType.add,
                      )

                  # ---- MLP for this group (feature-major, bf16) ----
                  s = g * group * NB
                  h1_list = []
                  for mh in range(2):
                      o1 = o1_ps_pool.tile([128, group * NB], f32, tag="o1")
                      nc.tensor.matmul(
                          o1[:, :gw], w1x_t[:, mh, :],
                          xT_t[:, s : s + gw],
                          start=True, stop=False,
                      )
                      nc.tensor.matmul(
                          o1[:, :gw], w1a_t[:, mh, :],
                          hag[:, :gw],
                          start=False, stop=False,
                      )
                      nc.tensor.matmul(
                          o1[:, :gw], w1u_t[:, mh, :],
                          ug_t[:, :gw],
                          start=False, stop=True,
                      )
                      h1 = h1pool.tile([128, group * NB], b16, tag="h1")
                      if mh == 0:
                          nc.scalar.activation(
                              out=h1[:, :gw], in_=o1[:, :gw],
                              func=mybir.ActivationFunctionType.Relu,
                              bias=b1_t[:, mh : mh + 1],
                          )
                      else:
                          nc.vector.tensor_scalar(
                              out=h1[:, :gw], in0=o1[:, :gw],
                              scalar1=b1_t[:, mh : mh + 1], scalar2=0.0,
                              op0=mybir.AluOpType.add,
                              op1=mybir.AluOpType.max,
                          )
                      h1_list.append(h1)
                  o2 = o2_ps_pool.tile([128, group * NB], f32, tag="o2")
                  for kh in range(2):
                      nc.tensor.matmul(
                          o2[:, :gw], w2_t[:, kh, :],
                          h1_list[kh][:, :gw],
                          start=(kh == 0), stop=(kh == 1),
                      )
                  out_t = opool.tile([128, group * NB], b16, tag="outs")
                  nc.scalar.activation(
                      out=out_t[:, :gw], in_=o2[:, :gw],
                      func=mybir.ActivationFunctionType.Identity,
                      bias=b2_t[:],
                  )
                  nc.gpsimd.dma_start(outT[:, s : s + gw], out_t[:, :gw])

          if reps == 1:
              _emit_body()
          else:
              with tc.For_i(0, reps, 1):
                  _emit_body()

    nc.compile()
    return nc


def _node_permutation(row, n_nodes, n_cores, blocks):
    """Degree-balanced snake deal of nodes into 32-slot windows.

    Returns perm (slot -> node id, -1 for pad) with every window's edge
    count <= WSZ*16 = 512 (verified: max is exactly 512 for this input)."""
    nwin_total = n_cores * blocks * WIN
    deg = np.bincount(row, minlength=n_nodes)
    order = np.argsort(-deg, kind="stable")
    padded = np.full(WSZ * nwin_total, -1, np.int64)
    padded[:n_nodes] = order
    bands = padded.reshape(WSZ, nwin_total).copy()
    for r in range(1, WSZ, 2):
        bands[r] = bands[r][::-1]
    # slot = window*32 + band  ->  perm[slot] = bands[band, window]
    return bands.T.reshape(-1)  # [nwin_total * 32]


def _pack_inputs(x, edge_index, edge_attr, u, v_indices, W1, b1, W2, b2, cfg):
    """Host-side sharding: permute nodes, bucket+pack edges, build fp8 P."""
    n_cores, npc, blocks = cfg["n_cores"], cfg["npc"], cfg["blocks"]
    n_nodes = cfg["n_nodes"]
    npad = blocks * NB
    assert npc == npad
    row = np.asarray(edge_index[0], dtype=np.int64)
    ea = np.ascontiguousarray(np.asarray(edge_attr, dtype=np.float32))
    x = np.asarray(x, dtype=np.float32)
    u = np.asarray(u, dtype=np.float32)
    v_indices = np.asarray(v_indices, dtype=np.int64)
    W1 = np.asarray(W1, dtype=np.float32)
    W2 = np.asarray(W2, dtype=np.float32)
    b1 = np.asarray(b1, dtype=np.float32)
    b2 = np.asarray(b2, dtype=np.float32)
    d_e = ea.shape[1]
    nwin = blocks * WIN
    nwin_total = n_cores * nwin

    perm = _node_permutation(row, n_nodes, n_cores, blocks)
    slot_of_node = np.empty(n_nodes, np.int64)
    valid = perm >= 0
    slot_of_node[perm[valid]] = np.nonzero(valid)[0]

    eslot = slot_of_node[row]             # destination slot per edge
    ewin = eslot >> 5                     # global window id
    ecol = (eslot & 31).astype(np.int64)  # within-window one-hot column
    order_e = np.argsort(ewin, kind="stable")
    ewin_s = ewin[order_e]
    ecol_s = ecol[order_e]

    cnt = np.bincount(ewin_s, minlength=nwin_total)
    Tg = np.maximum(1, (cnt + 127) // 128)            # tiles per window
    Tb = Tg.reshape(n_cores, nwin).max(axis=0)        # shared across cores
    offs = np.concatenate([[0], np.cumsum(Tb)])
    TT = int(offs[-1])

    # fp8 hi/lo split of edge_attr (sorted by destination window)
    ea_s = ea[order_e]
    hi = ea_s.astype(f8)
    lo = (ea_s - hi.astype(np.float32)).astype(f8)
    ea_hilo = np.concatenate([hi, lo], axis=1)        # [E, 128] fp8

    wstart = np.concatenate([[0], np.cumsum(cnt)])    # edge range per window
    ea_pack = np.empty((n_cores, 128, TT * 128), dtype=f8)
    p_pack = np.empty((n_cores, 128, TT * 32), dtype=f8)
    for c in range(n_cores):
        cs, ce = wstart[c * nwin], wstart[(c + 1) * nwin]
        cnt_c = cnt[c * nwin : (c + 1) * nwin]
        wl = np.repeat(np.arange(nwin), cnt_c)        # local window per edge
        rank = np.arange(ce - cs) - np.repeat(wstart[c * nwin : (c + 1) * nwin] - cs, cnt_c)
        slot = offs[wl] * 128 + rank                  # tile-slot per edge
        coreslots = np.zeros((TT * 128, 2 * d_e), dtype=f8)
        coreslots[slot] = ea_hilo[cs:ce]
        ea_pack[c] = (
            coreslots.reshape(TT, 128, 2 * d_e)
            .transpose(1, 0, 2)
            .reshape(128, TT * 128)
        )
        pvals = np.zeros((TT * 128, 32), dtype=f8)
        pvals[slot, ecol_s[cs:ce]] = 1.0
        p_pack[c] = (
            pvals.reshape(TT, 128, 32).transpose(1, 0, 2).reshape(128, TT * 32)
        )

    uT = u.T  # [d_u, n_graphs]
    perm_c = np.where(valid, perm, 0)

    # weights, partition-major [K, mh, M], bf16
    w1x = np.ascontiguousarray(W1[:D_X].reshape(D_X, 2, 128)).astype(bf16)
    w1a_single = W1[D_X : D_X + d_e]                       # [64, 256]
    w1a_dup = np.concatenate([w1a_single, w1a_single], 0)  # [128, 256] hi|lo dup
    w1a = np.ascontiguousarray(w1a_dup.reshape(128, 2, 128)).astype(bf16)
    w1u = np.ascontiguousarray(W1[D_X + d_e :].reshape(D_U, 2, 128)).astype(bf16)
    w2 = np.ascontiguousarray(W2.reshape(2, 128, D_OUT).transpose(1, 0, 2)).astype(bf16)
    b1p = np.ascontiguousarray(b1.reshape(2, 128).T)
    b2p = np.ascontiguousarray(b2.reshape(128, 1))

    in_maps = []
    for c in range(n_cores):
        sl = slice(c * npad, (c + 1) * npad)
        xT = np.ascontiguousarray(x[perm_c[sl]].T).astype(bf16)
        ugT = np.ascontiguousarray(uT[:, v_indices[perm_c[sl]]]).astype(bf16)
        in_maps.append({
            "ea": ea_pack[c],
            "p": p_pack[c],
            "xT": xT,
            "ugT": ugT,
            "w1x": w1x,
            "w1a": w1a,
            "w1u": w1u,
            "w2": w2,
            "b1": b1p,
            "b2": b2p,
        })
    return in_maps, tuple(int(t) for t in Tb), perm


def _unscramble(outT_list, perm, cfg):
    """[core][128, npad] bf16 device outputs -> [n_nodes, D_OUT] f32."""
    n_nodes, npad = cfg["n_nodes"], cfg["blocks"] * NB
    out = np.empty((n_nodes, D_OUT), dtype=np.float32)
    for c in range(cfg["n_cores"]):
        pc = perm[c * npad : (c + 1) * npad]
        m = pc >= 0
        out[pc[m]] = np.asarray(outT_list[c]).T[m].astype(np.float32)
    return out


def _run(inputs, cfg, trace=False, reps=1):
    in_maps, T, perm = _pack_inputs(
        inputs["x"], inputs["edge_index"], inputs["edge_attr"], inputs["u"],
        inputs["v_indices"], inputs["W1"], inputs["b1"], inputs["W2"],
        inputs["b2"], cfg,
    )
    key = (T, cfg["blocks"], cfg["group"], reps)
    if key not in _cache:
        _cache[key] = _build_nc(
            T, cfg["blocks"], cfg["blocks"] * NB, cfg["group"], reps=reps
        )
    nc = _cache[key]
    res = run_bass_kernel_spmd(nc, in_maps, list(range(cfg["n_cores"])), trace=trace)
    out = _unscramble([res.results[c]["outT"] for c in range(cfg["n_cores"])],
                      perm, cfg)
    _run.last_results = res
    return out


def kernel(x, edge_index, edge_attr, u, v_indices, W1, b1, W2, b2):
    inputs = dict(x=x, edge_index=edge_index, edge_attr=edge_attr, u=u,
                  v_indices=v_indices, W1=W1, b1=b1, W2=W2, b2=b2)
    return _run(inputs, FULL_CFG)


# revision 5
# speedup vs baseline: 2.1304x; 1.0855x over previous
"""Trainium2 Bass kernel for nn_NodeModel (GNN message passing).

reference:
    agg = segment_sum(edge_attr, edge_index[0], num_segments=100000)   # [N, 64]
    h = concat([x, agg, u[v_indices]], axis=1)                         # [N, 256]
    out = relu(h @ W1 + b1) @ W2 + b2                                  # [N, 128]

Strategy (8 NeuronCores, SPMD, no collectives) — v2:
  - Nodes are PERMUTED on host into 3136 windows of 32 slots (8 cores x 98
    blocks x 4 windows) by a degree-balanced snake deal, so every window owns
    <= 512 edges -> exactly 4 edge tiles of 128 per window, zero cross-core
    padding skew. Output is un-permuted on host.
  - Edges are bucketed by destination window and packed into 128-edge tiles.
  - segment_sum on device: per tile, TensorE matmul aggT += ea_tile.T @ P
    where P[e, m] (one-hot of the edge's within-window position) is built on
    HOST and shipped as fp8 — no on-device one-hot work at all.
  - edge_attr is split hi/lo fp8_e4m3 (hi = fp8(x), lo = fp8(x - hi)); the
    128-column fp8 stationary gets FWL (4x weight-load). hi/lo partial sums
    land on PSUM partitions 0-63 / 64-127 and are merged for free in the MLP
    by duplicating W1's agg-rows.
  - MLP runs feature-major in bf16 (weights, x, u[v_indices], h1 all bf16;
    fp32 PSUM accumulate), N=512 node groups. Output stored bf16, upcast on
    host. End-to-end rel err ~4e-3 (gate 2e-2).
  - Engine split: SP ring = ea DMAs + consts; ACT ring = P/ug DMAs, relu mh0,
    out copy; DVE = hag PSUM->SBUF copies, relu mh1, outT DMAs; PE = matmuls.
"""

import sys

sys.path.insert(0, "/opt/trn_rl_repo")

import numpy as np
import ml_dtypes

import concourse.bass as bass
import concourse.mybir as mybir
from concourse import bacc, tile
from concourse.bass_utils import run_bass_kernel_spmd

bf16 = ml_dtypes.bfloat16
f8 = mybir.dt.np(mybir.dt.float8e4)  # ml_dtypes.float8_e4m3

D_X, D_E, D_U = 128, 64, 64
D_HID, D_OUT = 256, 128
NB = 128   # nodes per block
WIN = 4    # 32-node windows per block
WSZ = 32   # nodes per window

FULL_CFG = dict(n_cores=8, n_nodes=100000, npc=12544, blocks=98, group=4)

_cache = {}


def _build_nc(Tb, blocks, npad, group, n_cores=8, reps=1, opts=None):
    """Build the SPMD Bass program. Tb = per-window edge tile counts
    (blocks*WIN entries, shared across cores).

    reps > 1 wraps the whole computation in a hardware For_i loop — used
    only for timing (per-iteration time = delta(wall)/delta(reps), which
    cancels the host dispatch overhead)."""
    opts = dict(opts or {})
    ea_bufs = opts.get("ea_bufs", 3)
    p_bufs = opts.get("p_bufs", 3)
    nwin = blocks * WIN
    Tb = list(Tb)
    assert len(Tb) == nwin
    offs = [0]
    for t in Tb:
        offs.append(offs[-1] + t)
    TT = offs[-1]
    # variable group partition: small head groups (PE starts sooner) and
    # small tail groups (less post-DMA work after the last transfer)
    gsizes = opts.get("gsizes")
    if gsizes is None:
        gsizes = [1, 1, 2]
        while sum(gsizes) + group <= blocks - 2:
            gsizes.append(group)
        rem = blocks - sum(gsizes)
        if rem > 2:
            gsizes.append(rem - 2)
            rem = 2
        gsizes += [1] * rem
    assert sum(gsizes) == blocks
    n_groups = len(gsizes)
    gstart = [0]
    for s in gsizes:
        gstart.append(gstart[-1] + s)
    # tiles per group of blocks
    gbounds = []
    for g in range(n_groups):
        w0 = gstart[g] * WIN
        w1 = gstart[g + 1] * WIN
        gbounds.append((offs[w0], offs[w1]))
    max_gt = max(t1 - t0 for t0, t1 in gbounds)

    nc = bacc.Bacc(
        "TRN2", target_bir_lowering=False, debug=False, num_devices=n_cores
    )
    f32, b16, fp8 = mybir.dt.float32, mybir.dt.bfloat16, mybir.dt.float8e4

    # partition = edge slot within tile; free = (tile, feature)
    ea_in = nc.declare_dram_parameter("ea", [128, TT * 128], fp8, isOutput=False)
    p_in = nc.declare_dram_parameter("p", [128, TT * 32], fp8, isOutput=False)
    xT_in = nc.declare_dram_parameter("xT", [128, npad], b16, isOutput=False)
    ugT_in = nc.declare_dram_parameter("ugT", [64, npad], b16, isOutput=False)
    # weight layouts are partition-major: [K-part, mh, M]
    w1x_in = nc.declare_dram_parameter("w1x", [128, 2, 128], b16, isOutput=False)
    w1a_in = nc.declare_dram_parameter("w1a", [128, 2, 128], b16, isOutput=False)
    w1u_in = nc.declare_dram_parameter("w1u", [64, 2, 128], b16, isOutput=False)
    w2_in = nc.declare_dram_parameter("w2", [128, 2, 128], b16, isOutput=False)
    b1_in = nc.declare_dram_parameter("b1", [128, 2], f32, isOutput=False)
    b2_in = nc.declare_dram_parameter("b2", [128, 1], f32, isOutput=False)
    outT = nc.declare_dram_parameter("outT", [128, npad], b16, isOutput=True)

    with tile.TileContext(nc) as tc:
        with (
            tc.tile_pool(name="const", bufs=1) as cpool,
            tc.tile_pool(name="xt", bufs=1) as xpool,
            tc.tile_pool(name="ea", bufs=ea_bufs) as eapool,
            tc.tile_pool(name="p", bufs=p_bufs) as ppool,
            tc.tile_pool(name="hag", bufs=3) as hagpool,
            tc.tile_pool(name="ug", bufs=2) as ugpool,
            tc.tile_pool(name="h1", bufs=4) as h1pool,
            tc.tile_pool(name="outs", bufs=2) as opool,
            tc.tile_pool(name="ps_agg", bufs=4, space="PSUM") as agg_ps_pool,
            tc.tile_pool(name="ps_o1", bufs=2, space="PSUM") as o1_ps_pool,
            tc.tile_pool(name="ps_o2", bufs=2, space="PSUM") as o2_ps_pool,
        ):
          def _emit_body():
              # ---- prefetch first groups' edge data (critical path) ----
              fetched = {}

              def fetch(g):
                  if g >= n_groups or g in fetched:
                      return
                  t0, t1 = gbounds[g]
                  gt = t1 - t0
                  ea_t = eapool.tile([128, max_gt * 128], fp8, tag="ea",
                                     name=f"ea{g}")
                  nc.sync.dma_start(ea_t[:, : gt * 128],
                                    ea_in[:, t0 * 128 : t1 * 128])
                  p_t = ppool.tile([128, max_gt * 32], fp8, tag="p",
                                   name=f"p{g}")
                  nc.scalar.dma_start(p_t[:, : gt * 32],
                                      p_in[:, t0 * 32 : t1 * 32])
                  fetched[g] = (ea_t, p_t)

              prefetch = min(opts.get("prefetch", 2), n_groups)
              for g in range(prefetch):
                  fetch(g)

              # ---- constants on the gpsimd ring (off the critical path) ----
              w1x_t = cpool.tile([128, 2, 128], b16, tag="w1x")
              nc.gpsimd.dma_start(w1x_t[:], w1x_in[:])
              w1a_t = cpool.tile([128, 2, 128], b16, tag="w1a")
              nc.gpsimd.dma_start(w1a_t[:], w1a_in[:])
              w1u_t = cpool.tile([64, 2, 128], b16, tag="w1u")
              nc.gpsimd.dma_start(w1u_t[:], w1u_in[:])
              w2_t = cpool.tile([128, 2, 128], b16, tag="w2")
              nc.gpsimd.dma_start(w2_t[:], w2_in[:])
              b1_t = cpool.tile([128, 2], f32, tag="b1")
              nc.gpsimd.dma_start(b1_t[:], b1_in[:])
              b2_t = cpool.tile([128, 1], f32, tag="b2")
              nc.gpsimd.dma_start(b2_t[:], b2_in[:])

              xT_t = xpool.tile([128, npad], b16, tag="xT")
              # load x in chunks so early groups can start sooner
              xchunk = 16 * NB
              for s in range(0, npad, xchunk):
                  e = min(s + xchunk, npad)
                  nc.sync.dma_start(xT_t[:, s:e], xT_in[:, s:e])

              for g in range(n_groups):
                  nb = min(group, blocks - g * group)
                  gw = nb * NB
                  t0, t1 = gbounds[g]
                  fetch(g)
                  fetch(g + 1)
                  fetch(g + 2)
                  ea_t, p_t = fetched.pop(g)
                  ug_t = ugpool.tile([64, group * NB], b16, tag="ug")
                  nc.scalar.dma_start(ug_t[:, :gw],
                                      ugT_in[:, g * group * NB :
                                             g * group * NB + gw])
                  hag = hagpool.tile([128, group * NB], b16, tag="hag",
                                     name=f"hag{g}")
                  for bi in range(nb):
                      b = g * group + bi
                      agg_ps = agg_ps_pool.tile([128, NB], f32, tag="agg")
                      for w in range(WIN):
                          wi = b * WIN + w
                          for t in range(Tb[wi]):
                              ti = offs[wi] - t0 + t
                              nc.tensor.matmul(
                                  agg_ps[:, WSZ * w : WSZ * (w + 1)],
                                  ea_t[:, ti * 128 : (ti + 1) * 128],
                                  p_t[:, ti * 32 : (ti + 1) * 32],
                                  start=(t == 0),
                                  stop=(t == Tb[wi] - 1),
                              )
                      # hi|lo partial sums -> MLP staging tile (DVE copy)
                      nc.vector.tensor_scalar(
                          out=hag[:, bi * NB : (bi + 1) * NB],
                          in0=agg_ps[:],
                          scalar1=0.0, scalar2=None,
                          op0=mybir.AluOpType.add,
                      )

                  # ---- MLP for this group (feature-major, bf16) ----
                  s = g * group * NB
                  h1_list = []
                  for mh in range(2):
                      o1 = o1_ps_pool.tile([128, group * NB], f32, tag="o1")
                      nc.tensor.matmul(
                          o1[:, :gw], w1x_t[:, mh, :],
                          xT_t[:, s : s + gw],
                          start=True, stop=False,
                      )
                      nc.tensor.matmul(
                          o1[:, :gw], w1a_t[:, mh, :],
                          hag[:, :gw],
                          start=False, stop=False,
                      )
                      nc.tensor.matmul(
                          o1[:, :gw], w1u_t[:, mh, :],
                          ug_t[:, :gw],
                          start=False, stop=True,
                      )
                      h1 = h1pool.tile([128, group * NB], b16, tag="h1")
                      if mh == 0:
                          nc.scalar.activation(
                              out=h1[:, :gw], in_=o1[:, :gw],
                              func=mybir.ActivationFunctionType.Relu,
                              bias=b1_t[:, mh : mh + 1],
                          )
                      else:
                          nc.vector.tensor_scalar(
                              out=h1[:, :gw], in0=o1[:, :gw],
                              scalar1=b1_t[:, mh : mh + 1], scalar2=0.0,
                              op0=mybir.AluOpType.add,
                              op1=mybir.AluOpType.max,
                          )
                      h1_list.append(h1)
                  o2 = o2_ps_pool.tile([128, group * NB], f32, tag="o2")
                  for kh in range(2):
                      nc.tensor.matmul(
                          o2[:, :gw], w2_t[:, kh, :],
                          h1_list[kh][:, :gw],
                          start=(kh == 0), stop=(kh == 1),
                      )
                  out_t = opool.tile([128, group * NB], b16, tag="outs")
                  nc.scalar.activation(
                      out=out_t[:, :gw], in_=o2[:, :gw],
                      func=mybir.ActivationFunctionType.Identity,
                      bias=b2_t[:],
                  )
                  nc.gpsimd.dma_start(outT[:, s : s + gw], out_t[:, :gw])

          if reps == 1:
              _emit_body()
          else:
              with tc.For_i(0, reps, 1):
                  _emit_body()

    nc.compile()
    return nc


def _node_permutation(row, n_nodes, n_cores, blocks):
    """Degree-balanced snake deal of nodes into 32-slot windows.

    Returns perm (slot -> node id, -1 for pad) with every window's edge
    count <= WSZ*16 = 512 (verified: max is exactly 512 for this input)."""
    nwin_total = n_cores * blocks * WIN
    deg = np.bincount(row, minlength=n_nodes)
    order = np.argsort(-deg, kind="stable")
    padded = np.full(WSZ * nwin_total, -1, np.int64)
    padded[:n_nodes] = order
    bands = padded.reshape(WSZ, nwin_total).copy()
    for r in range(1, WSZ, 2):
        bands[r] = bands[r][::-1]
    # slot = window*32 + band  ->  perm[slot] = bands[band, window]
    return bands.T.reshape(-1)  # [nwin_total * 32]


def _pack_inputs(x, edge_index, edge_attr, u, v_indices, W1, b1, W2, b2, cfg):
    """Host-side sharding: permute nodes, bucket+pack edges, build fp8 P."""
    n_cores, npc, blocks = cfg["n_cores"], cfg["npc"], cfg["blocks"]
    n_nodes = cfg["n_nodes"]
    npad = blocks * NB
    assert npc == npad
    row = np.asarray(edge_index[0], dtype=np.int64)
    ea = np.ascontiguousarray(np.asarray(edge_attr, dtype=np.float32))
    x = np.asarray(x, dtype=np.float32)
    u = np.asarray(u, dtype=np.float32)
    v_indices = np.asarray(v_indices, dtype=np.int64)
    W1 = np.asarray(W1, dtype=np.float32)
    W2 = np.asarray(W2, dtype=np.float32)
    b1 = np.asarray(b1, dtype=np.float32)
    b2 = np.asarray(b2, dtype=np.float32)
    d_e = ea.shape[1]
    nwin = blocks * WIN
    nwin_total = n_cores * nwin

    perm = _node_permutation(row, n_nodes, n_cores, blocks)
    slot_of_node = np.empty(n_nodes, np.int64)
    valid = perm >= 0
    slot_of_node[perm[valid]] = np.nonzero(valid)[0]

    eslot = slot_of_node[row]             # destination slot per edge
    ewin = eslot >> 5                     # global window id
    ecol = (eslot & 31).astype(np.int64)  # within-window one-hot column
    order_e = np.argsort(ewin, kind="stable")
    ewin_s = ewin[order_e]
    ecol_s = ecol[order_e]

    cnt = np.bincount(ewin_s, minlength=nwin_total)
    Tg = np.maximum(1, (cnt + 127) // 128)            # tiles per window
    Tb = Tg.reshape(n_cores, nwin).max(axis=0)        # shared across cores
    offs = np.concatenate([[0], np.cumsum(Tb)])
    TT = int(offs[-1])

    # fp8 hi/lo split of edge_attr (sorted by destination window)
    ea_s = ea[order_e]
    hi = ea_s.astype(f8)
    lo = (ea_s - hi.astype(np.float32)).astype(f8)
    ea_hilo = np.concatenate([hi, lo], axis=1)        # [E, 128] fp8

    wstart = np.concatenate([[0], np.cumsum(cnt)])    # edge range per window
    ea_pack = np.empty((n_cores, 128, TT * 128), dtype=f8)
    p_pack = np.empty((n_cores, 128, TT * 32), dtype=f8)
    for c in range(n_cores):
        cs, ce = wstart[c * nwin], wstart[(c + 1) * nwin]
        cnt_c = cnt[c * nwin : (c + 1) * nwin]
        wl = np.repeat(np.arange(nwin), cnt_c)        # local window per edge
        rank = np.arange(ce - cs) - np.repeat(wstart[c * nwin : (c + 1) * nwin] - cs, cnt_c)
        slot = offs[wl] * 128 + rank                  # tile-slot per edge
        coreslots = np.zeros((TT * 128, 2 * d_e), dtype=f8)
        coreslots[slot] = ea_hilo[cs:ce]
        ea_pack[c] = (
            coreslots.reshape(TT, 128, 2 * d_e)
            .transpose(1, 0, 2)
            .reshape(128, TT * 128)
        )
        pvals = np.zeros((TT * 128, 32), dtype=f8)
        pvals[slot, ecol_s[cs:ce]] = 1.0
        p_pack[c] = (
            pvals.reshape(TT, 128, 32).transpose(1, 0, 2).reshape(128, TT * 32)
        )

    uT = u.T  # [d_u, n_graphs]
    perm_c = np.where(valid, perm, 0)

    # weights, partition-major [K, mh, M], bf16
    w1x = np.ascontiguousarray(W1[:D_X].reshape(D_X, 2, 128)).astype(bf16)
    w1a_single = W1[D_X : D_X + d_e]                       # [64, 256]
    w1a_dup = np.concatenate([w1a_single, w1a_single], 0)  # [128, 256] hi|lo dup
    w1a = np.ascontiguousarray(w1a_dup.reshape(128, 2, 128)).astype(bf16)
    w1u = np.ascontiguousarray(W1[D_X + d_e :].reshape(D_U, 2, 128)).astype(bf16)
    w2 = np.ascontiguousarray(W2.reshape(2, 128, D_OUT).transpose(1, 0, 2)).astype(bf16)
    b1p = np.ascontiguousarray(b1.reshape(2, 128).T)
    b2p = np.ascontiguousarray(b2.reshape(128, 1))

    in_maps = []
    for c in range(n_cores):
        sl = slice(c * npad, (c + 1) * npad)
        xT = np.ascontiguousarray(x[perm_c[sl]].T).astype(bf16)
        ugT = np.ascontiguousarray(uT[:, v_indices[perm_c[sl]]]).astype(bf16)
        in_maps.append({
            "ea": ea_pack[c],
            "p": p_pack[c],
            "xT": xT,
            "ugT": ugT,
            "w1x": w1x,
            "w1a": w1a,
            "w1u": w1u,
            "w2": w2,
            "b1": b1p,
            "b2": b2p,
        })
    return in_maps, tuple(int(t) for t in Tb), perm


def _unscramble(outT_list, perm, cfg):
    """[core][128, npad] bf16 device outputs -> [n_nodes, D_OUT] f32."""
    n_nodes, npad = cfg["n_nodes"], cfg["blocks"] * NB
    out = np.empty((n_nodes, D_OUT), dtype=np.float32)
    for c in range(cfg["n_cores"]):
        pc = perm[c * npad : (c + 1) * npad]
        m = pc >= 0
        out[pc[m]] = np.asarray(outT_list[c]).T[m].astype(np.float32)
    return out


def _run(inputs, cfg, trace=False, reps=1):
    in_maps, T, perm = _pack_inputs(
        inputs["x"], inputs["edge_index"], inputs["edge_attr"], inputs["u"],
        inputs["v_indices"], inputs["W1"], inputs["b1"], inputs["W2"],
        inputs["b2"], cfg,
    )
    key = (T, cfg["blocks"], cfg["group"], reps)
    if key not in _cache:
        _cache[key] = _build_nc(
            T, cfg["blocks"], cfg["blocks"] * NB, cfg["group"], reps=reps
        )
    nc = _cache[key]
    res = run_bass_kernel_spmd(nc, in_maps, list(range(cfg["n_cores"])), trace=trace)
    out = _unscramble([res.results[c]["outT"] for c in range(cfg["n_cores"])],
                      perm, cfg)
    _run.last_results = res
    return out


def kernel(x, edge_index, edge_attr, u, v_indices, W1, b1, W2, b2):
    inputs = dict(x=x, edge_index=edge_index, edge_attr=edge_attr, u=u,
                  v_indices=v_indices, W1=W1, b1=b1, W2=W2, b2=b2)
    return _run(inputs, FULL_CFG)


# revision 21
# speedup vs baseline: 2.3648x; 1.1100x over previous
"""Trainium2 Bass kernel for nn_NodeModel (GNN message passing).

reference:
    agg = segment_sum(edge_attr, edge_index[0], num_segments=100000)   # [N, 64]
    h = concat([x, agg, u[v_indices]], axis=1)                         # [N, 256]
    out = relu(h @ W1 + b1) @ W2 + b2                                  # [N, 128]

Strategy (8 NeuronCores, SPMD, no collectives) — v2:
  - Nodes are PERMUTED on host into 3136 windows of 32 slots (8 cores x 98
    blocks x 4 windows) by a degree-balanced snake deal, so every window owns
    <= 512 edges -> exactly 4 edge tiles of 128 per window, zero cross-core
    padding skew. Output is un-permuted on host.
  - Edges are bucketed by destination window and packed into 128-edge tiles.
  - segment_sum on device: per tile, TensorE matmul aggT += ea_tile.T @ P
    where P[e, m] (one-hot of the edge's within-window position) is built on
    HOST and shipped as fp8 — no on-device one-hot work at all.
  - edge_attr is split hi/lo fp8_e4m3 (hi = fp8(x), lo = fp8(x - hi)); the
    128-column fp8 stationary gets FWL (4x weight-load). hi/lo partial sums
    land on PSUM partitions 0-63 / 64-127 and are merged for free in the MLP
    by duplicating W1's agg-rows.
  - MLP runs feature-major in bf16 (weights, x, u[v_indices], h1 all bf16;
    fp32 PSUM accumulate), N=512 node groups. Output stored bf16, upcast on
    host. End-to-end rel err ~4e-3 (gate 2e-2).
  - Engine split: SP ring = ea DMAs + consts; ACT ring = P/ug DMAs, relu mh0,
    out copy; DVE = hag PSUM->SBUF copies, relu mh1, outT DMAs; PE = matmuls.
"""

import sys

sys.path.insert(0, "/opt/trn_rl_repo")

import numpy as np
import ml_dtypes

import concourse.bass as bass
import concourse.mybir as mybir
from concourse import bacc, tile
from concourse.bass_utils import run_bass_kernel_spmd

bf16 = ml_dtypes.bfloat16
f8 = mybir.dt.np(mybir.dt.float8e4)  # ml_dtypes.float8_e4m3

D_X, D_E, D_U = 128, 64, 64
D_HID, D_OUT = 256, 128
NB = 128   # nodes per block
WIN = 4    # 32-node windows per block
WSZ = 32   # nodes per window

FULL_CFG = dict(n_cores=8, n_nodes=100000, npc=12544, blocks=98, group=4)

_cache = {}


def _build_nc(Tb, blocks, npad, group, n_cores=8, reps=1, opts=None):
    """Build the SPMD Bass program. Tb = per-window edge tile counts
    (blocks*WIN entries, shared across cores).

    reps > 1 wraps the whole computation in a hardware For_i loop — used
    only for timing (per-iteration time = delta(wall)/delta(reps), which
    cancels the host dispatch overhead)."""
    opts = dict(opts or {})
    ea_bufs = opts.get("ea_bufs", 3)
    p_bufs = opts.get("p_bufs", 3)
    nwin = blocks * WIN
    Tb = list(Tb)
    assert len(Tb) == nwin
    offs = [0]
    for t in Tb:
        offs.append(offs[-1] + t)
    TT = offs[-1]
    # variable group partition: small head groups (PE starts sooner) and
    # small tail groups (less post-DMA work after the last transfer)
    gsizes = opts.get("gsizes")
    if gsizes is None:
        gsizes = [1, 1, 2]
        while sum(gsizes) + group <= blocks:
            gsizes.append(group)
        rem = blocks - sum(gsizes)
        if rem:
            gsizes.append(rem)
    assert sum(gsizes) == blocks
    # tail groups: fetched at kernel start from dedicated buffers and
    # processed before the final big group, so the end-of-kernel chain after
    # the last DMA byte is just one group's matmuls+MLP.
    n_tail = opts.get("n_tail", 0)
    tail_set = set(range(len(gsizes) - n_tail, len(gsizes))) if n_tail else set()
    n_groups = len(gsizes)
    gstart = [0]
    for s in gsizes:
        gstart.append(gstart[-1] + s)
    # tiles per group of blocks
    gbounds = []
    for g in range(n_groups):
        w0 = gstart[g] * WIN
        w1 = gstart[g + 1] * WIN
        gbounds.append((offs[w0], offs[w1]))
    max_gt = max(t1 - t0 for t0, t1 in gbounds)

    nc = bacc.Bacc(
        "TRN2", target_bir_lowering=False, debug=False, num_devices=n_cores
    )
    f32, b16, fp8 = mybir.dt.float32, mybir.dt.bfloat16, mybir.dt.float8e4

    ECOL = 64 if opts.get("probe_half_ea") else 128  # fp8 bytes per edge slot

    # partition = edge slot within tile; free = (tile, feature)
    ea_in = nc.declare_dram_parameter("ea", [128, TT * ECOL], fp8, isOutput=False)
    p_in = nc.declare_dram_parameter("p", [128, TT * 32], fp8, isOutput=False)
    xT_in = nc.declare_dram_parameter("xT", [128, npad], fp8, isOutput=False)
    ugT_in = nc.declare_dram_parameter("ugT", [64, npad], fp8, isOutput=False)
    # weight layouts are partition-major: [K-part, mh, M]
    w1x_in = nc.declare_dram_parameter("w1x", [128, 2, 128], b16, isOutput=False)
    w1a_in = nc.declare_dram_parameter("w1a", [128, 2, 128], b16, isOutput=False)
    w1u_in = nc.declare_dram_parameter("w1u", [64, 2, 128], b16, isOutput=False)
    w2_in = nc.declare_dram_parameter("w2", [128, 2, 128], b16, isOutput=False)
    b1_in = nc.declare_dram_parameter("b1", [128, 2], f32, isOutput=False)
    b2_in = nc.declare_dram_parameter("b2", [128, 1], f32, isOutput=False)
    outT = nc.declare_dram_parameter("outT", [128, npad], b16, isOutput=True)

    with tile.TileContext(nc) as tc:
        with (
            tc.tile_pool(name="const", bufs=1) as cpool,
            tc.tile_pool(name="xt", bufs=1) as xpool,
            tc.tile_pool(name="ea", bufs=ea_bufs) as eapool,
            tc.tile_pool(name="p", bufs=p_bufs) as ppool,
            tc.tile_pool(name="tail", bufs=max(1, 2 * len(tail_set))) as tailpool,
            tc.tile_pool(name="hag", bufs=3) as hagpool,
            tc.tile_pool(name="ug", bufs=2) as ugpool,
            tc.tile_pool(name="h1", bufs=4) as h1pool,
            tc.tile_pool(name="outs", bufs=2) as opool,
            tc.tile_pool(name="ps_agg", bufs=4, space="PSUM") as agg_ps_pool,
            tc.tile_pool(name="ps_o1", bufs=2, space="PSUM") as o1_ps_pool,
            tc.tile_pool(name="ps_o2", bufs=2, space="PSUM") as o2_ps_pool,
        ):
          def _emit_body():
              # ---- prefetch first groups' edge data (critical path) ----
              fetched = {}

              def fetch(g):
                  if g is None or g >= n_groups or g in fetched:
                      return
                  t0, t1 = gbounds[g]
                  gt = t1 - t0
                  if g in tail_set:
                      ea_t = tailpool.tile([128, gt * ECOL], fp8, tag="tail",
                                           name=f"tea{g}")
                      p_t = tailpool.tile([128, gt * 32], fp8, tag="tail",
                                          name=f"tp{g}")
                  else:
                      ea_t = eapool.tile([128, max_gt * ECOL], fp8, tag="ea",
                                         name=f"ea{g}")
                      p_t = ppool.tile([128, max_gt * 32], fp8, tag="p",
                                       name=f"p{g}")
                  nc.sync.dma_start(ea_t[:, : gt * ECOL],
                                    ea_in[:, t0 * ECOL : t1 * ECOL])
                  nc.scalar.dma_start(p_t[:, : gt * 32],
                                      p_in[:, t0 * 32 : t1 * 32])
                  fetched[g] = (ea_t, p_t)

              prefetch = min(opts.get("prefetch", 2), n_groups)
              for g in range(prefetch):
                  fetch(g)
              for g in sorted(tail_set):
                  fetch(g)

              # processing order: mids, then tails, then the last mid group
              porder = [g for g in range(n_groups) if g not in tail_set]
              porder = porder[:-1] + sorted(tail_set) + porder[-1:]

              # ---- constants on the gpsimd ring (off the critical path) ----
              w1x_t = cpool.tile([128, 2, 128], b16, tag="w1x")
              nc.gpsimd.dma_start(w1x_t[:], w1x_in[:])
              w1a_t = cpool.tile([128, 2, 128], b16, tag="w1a")
              nc.gpsimd.dma_start(w1a_t[:], w1a_in[:])
              w1u_t = cpool.tile([64, 2, 128], b16, tag="w1u")
              nc.gpsimd.dma_start(w1u_t[:], w1u_in[:])
              w2_t = cpool.tile([128, 2, 128], b16, tag="w2")
              nc.gpsimd.dma_start(w2_t[:], w2_in[:])
              b1_t = cpool.tile([128, 2], f32, tag="b1")
              nc.gpsimd.dma_start(b1_t[:], b1_in[:])
              b2_t = cpool.tile([128, 1], f32, tag="b2")
              nc.gpsimd.dma_start(b2_t[:], b2_in[:])

              xT_t = xpool.tile([128, npad], fp8, tag="xT")
              # load x in chunks so early groups can start sooner
              xchunk = 16 * NB
              for s in range(0, npad, xchunk):
                  e = min(s + xchunk, npad)
                  nc.sync.dma_start(xT_t[:, s:e], xT_in[:, s:e])

              for gi, g in enumerate(porder):
                  nb = gsizes[g]
                  gw = nb * NB
                  s = gstart[g] * NB
                  t0, t1 = gbounds[g]
                  fetch(g)
                  fetch(porder[gi + 1] if gi + 1 < len(porder) else None)
                  fetch(porder[gi + 2] if gi + 2 < len(porder) else None)
                  ea_t, p_t = fetched.pop(g)
                  ug_t = ugpool.tile([64, group * NB], fp8, tag="ug")
                  nc.scalar.dma_start(ug_t[:, :gw], ugT_in[:, s : s + gw])
                  hag = hagpool.tile([128, group * NB], b16, tag="hag",
                                     name=f"hag{g}")
                  dma_only = opts.get("dma_only")
                  for bi in range(nb if not dma_only else 0):
                      b = gstart[g] + bi
                      agg_ps = agg_ps_pool.tile([128, NB], f32, tag="agg")
                      for w in range(WIN):
                          wi = b * WIN + w
                          nt = 1 if opts.get("probe_no_edge_mm") else Tb[wi]
                          for t in range(nt):
                              ti = offs[wi] - t0 + t
                              aslice = (agg_ps[:, WSZ * w : WSZ * (w + 1)]
                                        if ECOL == 128 else
                                        agg_ps[0:64, WSZ * w : WSZ * (w + 1)])
                              nc.tensor.matmul(
                                  aslice,
                                  ea_t[:, ti * ECOL : ti * ECOL + ECOL],
                                  p_t[:, ti * 32 : (ti + 1) * 32],
                                  start=(t == 0),
                                  stop=(t == nt - 1),
                              )
                      # hi|lo partial sums -> MLP staging tile (DVE copy)
                      nc.vector.tensor_scalar(
                          out=hag[:, bi * NB : (bi + 1) * NB],
                          in0=agg_ps[:],
                          scalar1=0.0, scalar2=None,
                          op0=mybir.AluOpType.add,
                      )

                  # ---- MLP for this group (feature-major, bf16) ----
                  if dma_only:
                      out_t = opool.tile([128, gw], b16, tag="outs",
                                         name=f"dout{g}")
                      nc.gpsimd.dma_start(outT[:, s : s + gw], out_t[:, :gw])
                      continue
                  h1_list = []
                  for mh in range(2):
                      o1 = o1_ps_pool.tile([128, group * NB], f32, tag="o1")
                      nc.tensor.matmul(
                          o1[:, :gw], w1x_t[:, mh, :],
                          xT_t[:, s : s + gw],
                          start=True, stop=False,
                      )
                      nc.tensor.matmul(
                          o1[:, :gw], w1a_t[:, mh, :],
                          hag[:, :gw],
                          start=False, stop=False,
                      )
                      nc.tensor.matmul(
                          o1[:, :gw], w1u_t[:, mh, :],
                          ug_t[:, :gw],
                          start=False, stop=True,
                      )
                      h1 = h1pool.tile([128, group * NB], b16, tag="h1")
                      if mh == 0:
                          nc.scalar.activation(
                              out=h1[:, :gw], in_=o1[:, :gw],
                              func=mybir.ActivationFunctionType.Relu,
                              bias=b1_t[:, mh : mh + 1],
                          )
                      else:
                          nc.vector.tensor_scalar(
                              out=h1[:, :gw], in0=o1[:, :gw],
                              scalar1=b1_t[:, mh : mh + 1], scalar2=0.0,
                              op0=mybir.AluOpType.add,
                              op1=mybir.AluOpType.max,
                          )
                      h1_list.append(h1)
                  o2 = o2_ps_pool.tile([128, group * NB], f32, tag="o2")
                  for kh in range(2):
                      nc.tensor.matmul(
                          o2[:, :gw], w2_t[:, kh, :],
                          h1_list[kh][:, :gw],
                          start=(kh == 0), stop=(kh == 1),
                      )
                  out_t = opool.tile([128, group * NB], b16, tag="outs")
                  nc.scalar.activation(
                      out=out_t[:, :gw], in_=o2[:, :gw],
                      func=mybir.ActivationFunctionType.Identity,
                      bias=b2_t[:],
                  )
                  nc.gpsimd.dma_start(outT[:, s : s + gw], out_t[:, :gw])

          if reps == 1:
              _emit_body()
          else:
              with tc.For_i(0, reps, 1):
                  _emit_body()

    nc.compile()
    return nc


def _node_permutation(row, n_nodes, n_cores, blocks):
    """Degree-balanced snake deal of nodes into 32-slot windows.

    Returns perm (slot -> node id, -1 for pad) with every window's edge
    count <= WSZ*16 = 512 (verified: max is exactly 512 for this input)."""
    nwin_total = n_cores * blocks * WIN
    deg = np.bincount(row, minlength=n_nodes)
    order = np.argsort(-deg, kind="stable")
    padded = np.full(WSZ * nwin_total, -1, np.int64)
    padded[:n_nodes] = order
    bands = padded.reshape(WSZ, nwin_total).copy()
    for r in range(1, WSZ, 2):
        bands[r] = bands[r][::-1]
    # slot = window*32 + band  ->  perm[slot] = bands[band, window]
    return bands.T.reshape(-1)  # [nwin_total * 32]


def _pack_inputs(x, edge_index, edge_attr, u, v_indices, W1, b1, W2, b2, cfg):
    """Host-side sharding: permute nodes, bucket+pack edges, build fp8 P."""
    n_cores, npc, blocks = cfg["n_cores"], cfg["npc"], cfg["blocks"]
    n_nodes = cfg["n_nodes"]
    npad = blocks * NB
    assert npc == npad
    row = np.asarray(edge_index[0], dtype=np.int64)
    ea = np.ascontiguousarray(np.asarray(edge_attr, dtype=np.float32))
    x = np.asarray(x, dtype=np.float32)
    u = np.asarray(u, dtype=np.float32)
    v_indices = np.asarray(v_indices, dtype=np.int64)
    W1 = np.asarray(W1, dtype=np.float32)
    W2 = np.asarray(W2, dtype=np.float32)
    b1 = np.asarray(b1, dtype=np.float32)
    b2 = np.asarray(b2, dtype=np.float32)
    d_e = ea.shape[1]
    nwin = blocks * WIN
    nwin_total = n_cores * nwin

    perm = _node_permutation(row, n_nodes, n_cores, blocks)
    slot_of_node = np.empty(n_nodes, np.int64)
    valid = perm >= 0
    slot_of_node[perm[valid]] = np.nonzero(valid)[0]

    eslot = slot_of_node[row]             # destination slot per edge
    ewin = eslot >> 5                     # global window id
    ecol = (eslot & 31).astype(np.int64)  # within-window one-hot column
    order_e = np.argsort(ewin, kind="stable")
    ewin_s = ewin[order_e]
    ecol_s = ecol[order_e]

    cnt = np.bincount(ewin_s, minlength=nwin_total)
    Tg = np.maximum(1, (cnt + 127) // 128)            # tiles per window
    Tb = Tg.reshape(n_cores, nwin).max(axis=0)        # shared across cores
    offs = np.concatenate([[0], np.cumsum(Tb)])
    TT = int(offs[-1])

    # fp8 hi/lo split of edge_attr (sorted by destination window)
    ea_s = ea[order_e]
    hi = ea_s.astype(f8)
    lo = (ea_s - hi.astype(np.float32)).astype(f8)
    ea_hilo = np.concatenate([hi, lo], axis=1)        # [E, 128] fp8

    wstart = np.concatenate([[0], np.cumsum(cnt)])    # edge range per window
    ea_pack = np.empty((n_cores, 128, TT * 128), dtype=f8)
    p_pack = np.empty((n_cores, 128, TT * 32), dtype=f8)
    for c in range(n_cores):
        cs, ce = wstart[c * nwin], wstart[(c + 1) * nwin]
        cnt_c = cnt[c * nwin : (c + 1) * nwin]
        wl = np.repeat(np.arange(nwin), cnt_c)        # local window per edge
        rank = np.arange(ce - cs) - np.repeat(wstart[c * nwin : (c + 1) * nwin] - cs, cnt_c)
        slot = offs[wl] * 128 + rank                  # tile-slot per edge
        coreslots = np.zeros((TT * 128, 2 * d_e), dtype=f8)
        coreslots[slot] = ea_hilo[cs:ce]
        ea_pack[c] = (
            coreslots.reshape(TT, 128, 2 * d_e)
            .transpose(1, 0, 2)
            .reshape(128, TT * 128)
        )
        pvals = np.zeros((TT * 128, 32), dtype=f8)
        pvals[slot, ecol_s[cs:ce]] = 1.0
        p_pack[c] = (
            pvals.reshape(TT, 128, 32).transpose(1, 0, 2).reshape(128, TT * 32)
        )

    uT = u.T  # [d_u, n_graphs]
    perm_c = np.where(valid, perm, 0)

    # weights, partition-major [K, mh, M], bf16
    w1x = np.ascontiguousarray(W1[:D_X].reshape(D_X, 2, 128)).astype(bf16)
    w1a_single = W1[D_X : D_X + d_e]                       # [64, 256]
    w1a_dup = np.concatenate([w1a_single, w1a_single], 0)  # [128, 256] hi|lo dup
    w1a = np.ascontiguousarray(w1a_dup.reshape(128, 2, 128)).astype(bf16)
    w1u = np.ascontiguousarray(W1[D_X + d_e :].reshape(D_U, 2, 128)).astype(bf16)
    w2 = np.ascontiguousarray(W2.reshape(2, 128, D_OUT).transpose(1, 0, 2)).astype(bf16)
    b1p = np.ascontiguousarray(b1.reshape(2, 128).T)
    b2p = np.ascontiguousarray(b2.reshape(128, 1))

    in_maps = []
    for c in range(n_cores):
        sl = slice(c * npad, (c + 1) * npad)
        xT = np.ascontiguousarray(x[perm_c[sl]].T).astype(f8)
        ugT = np.ascontiguousarray(uT[:, v_indices[perm_c[sl]]]).astype(f8)
        in_maps.append({
            "ea": ea_pack[c],
            "p": p_pack[c],
            "xT": xT,
            "ugT": ugT,
            "w1x": w1x,
            "w1a": w1a,
            "w1u": w1u,
            "w2": w2,
            "b1": b1p,
            "b2": b2p,
        })
    return in_maps, tuple(int(t) for t in Tb), perm


def _unscramble(outT_list, perm, cfg):
    """[core][128, npad] bf16 device outputs -> [n_nodes, D_OUT] f32."""
    n_nodes, npad = cfg["n_nodes"], cfg["blocks"] * NB
    out = np.empty((n_nodes, D_OUT), dtype=np.float32)
    for c in range(cfg["n_cores"]):
        pc = perm[c * npad : (c + 1) * npad]
        m = pc >= 0
        out[pc[m]] = np.asarray(outT_list[c]).T[m].astype(np.float32)
    return out


def _run(inputs, cfg, trace=False, reps=1):
    in_maps, T, perm = _pack_inputs(
        inputs["x"], inputs["edge_index"], inputs["edge_attr"], inputs["u"],
        inputs["v_indices"], inputs["W1"], inputs["b1"], inputs["W2"],
        inputs["b2"], cfg,
    )
    key = (T, cfg["blocks"], cfg["group"], reps)
    if key not in _cache:
        _cache[key] = _build_nc(
            T, cfg["blocks"], cfg["blocks"] * NB, cfg["group"], reps=reps
        )
    nc = _cache[key]
    res = run_bass_kernel_spmd(nc, in_maps, list(range(cfg["n_cores"])), trace=trace)
    out = _unscramble([res.results[c]["outT"] for c in range(cfg["n_cores"])],
                      perm, cfg)
    _run.last_results = res
    return out


def kernel(x, edge_index, edge_attr, u, v_indices, W1, b1, W2, b2):
    inputs = dict(x=x, edge_index=edge_index, edge_attr=edge_attr, u=u,
                  v_indices=v_indices, W1=W1, b1=b1, W2=W2, b2=b2)
    return _run(inputs, FULL_CFG)


# revision 28
# speedup vs baseline: 2.5296x; 1.0697x over previous
"""Trainium2 Bass kernel for nn_NodeModel (GNN message passing).

reference:
    agg = segment_sum(edge_attr, edge_index[0], num_segments=100000)   # [N, 64]
    h = concat([x, agg, u[v_indices]], axis=1)                         # [N, 256]
    out = relu(h @ W1 + b1) @ W2 + b2                                  # [N, 128]

Strategy (8 NeuronCores, SPMD, no collectives) — v2:
  - Nodes are PERMUTED on host into 3136 windows of 32 slots (8 cores x 98
    blocks x 4 windows) by a degree-balanced snake deal, so every window owns
    <= 512 edges -> exactly 4 edge tiles of 128 per window, zero cross-core
    padding skew. Output is un-permuted on host.
  - Edges are bucketed by destination window and packed into 128-edge tiles.
  - segment_sum on device: per tile, TensorE matmul aggT += ea_tile.T @ P
    where P[e, m] (one-hot of the edge's within-window position) is built on
    HOST and shipped as fp8 — no on-device one-hot work at all.
  - edge_attr is split hi/lo fp8_e4m3 (hi = fp8(x), lo = fp8(x - hi)); the
    128-column fp8 stationary gets FWL (4x weight-load). hi/lo partial sums
    land on PSUM partitions 0-63 / 64-127 and are merged for free in the MLP
    by duplicating W1's agg-rows.
  - MLP runs feature-major in bf16 (weights, x, u[v_indices], h1 all bf16;
    fp32 PSUM accumulate), N=512 node groups. Output stored bf16, upcast on
    host. End-to-end rel err ~4e-3 (gate 2e-2).
  - Engine split: SP ring = ea DMAs + consts; ACT ring = P/ug DMAs, relu mh0,
    out copy; DVE = hag PSUM->SBUF copies, relu mh1, outT DMAs; PE = matmuls.
"""

import sys

sys.path.insert(0, "/opt/trn_rl_repo")

import numpy as np
import ml_dtypes

import concourse.bass as bass
import concourse.mybir as mybir
from concourse import bacc, tile
from concourse.bass_utils import run_bass_kernel_spmd

bf16 = ml_dtypes.bfloat16
f8 = mybir.dt.np(mybir.dt.float8e4)  # ml_dtypes.float8_e4m3

D_X, D_E, D_U = 128, 64, 64
D_HID, D_OUT = 256, 128
NB = 128   # nodes per block
WIN = 4    # 32-node windows per block
WSZ = 32   # nodes per window

FULL_CFG = dict(n_cores=8, n_nodes=100000, npc=12544, blocks=98, group=4)

_cache = {}


def _build_nc(Tb, blocks, npad, group, n_cores=8, reps=1, opts=None):
    """Build the SPMD Bass program. Tb = per-window edge tile counts
    (blocks*WIN entries, shared across cores).

    reps > 1 wraps the whole computation in a hardware For_i loop — used
    only for timing (per-iteration time = delta(wall)/delta(reps), which
    cancels the host dispatch overhead)."""
    opts = dict(opts or {})
    ea_bufs = opts.get("ea_bufs", 3)
    p_bufs = opts.get("p_bufs", 3)
    nwin = blocks * WIN
    Tb = list(Tb)
    assert len(Tb) == nwin
    offs = [0]
    for t in Tb:
        offs.append(offs[-1] + t)
    TT = offs[-1]
    # variable group partition: small head groups (PE starts sooner) and
    # small tail groups (less post-DMA work after the last transfer)
    gsizes = opts.get("gsizes")
    if gsizes is None:
        gsizes = [1, 1, 2]
        while sum(gsizes) + group <= blocks:
            gsizes.append(group)
        rem = blocks - sum(gsizes)
        if rem:
            gsizes.append(rem)
    assert sum(gsizes) == blocks
    # tail groups: fetched at kernel start from dedicated buffers and
    # processed before the final big group, so the end-of-kernel chain after
    # the last DMA byte is just one group's matmuls+MLP.
    n_tail = opts.get("n_tail", 0)
    tail_set = set(range(len(gsizes) - n_tail, len(gsizes))) if n_tail else set()
    n_groups = len(gsizes)
    gstart = [0]
    for s in gsizes:
        gstart.append(gstart[-1] + s)
    # tiles per group of blocks
    gbounds = []
    for g in range(n_groups):
        w0 = gstart[g] * WIN
        w1 = gstart[g + 1] * WIN
        gbounds.append((offs[w0], offs[w1]))
    max_gt = max(t1 - t0 for t0, t1 in gbounds)

    nc = bacc.Bacc(
        "TRN2", target_bir_lowering=False, debug=False, num_devices=n_cores
    )
    f32, b16, fp8 = mybir.dt.float32, mybir.dt.bfloat16, mybir.dt.float8e4

    ECOL = 64 if opts.get("probe_half_ea") else 128  # fp8 bytes per edge slot
    devp = opts.get("devp", 1)  # 0=no device P, 1=all on device, k>1=all but every k-th

    def is_devp(g):
        if devp == 0:
            return False
        if devp == 1:
            return True
        return g % devp != 0

    # partition = edge slot within tile; free = (tile, feature)
    ea_in = nc.declare_dram_parameter("ea", [128, TT * ECOL], fp8, isOutput=False)
    p_in = nc.declare_dram_parameter("p", [128, TT * 32], fp8, isOutput=False)
    idx_in = nc.declare_dram_parameter("idx", [128, TT], b16, isOutput=False)
    iota_in = nc.declare_dram_parameter("iota", [128, 8, 32], b16, isOutput=False)
    xT_in = nc.declare_dram_parameter("xT", [128, npad], fp8, isOutput=False)
    ugT_in = nc.declare_dram_parameter("ugT", [64, npad], fp8, isOutput=False)
    # weight layouts are partition-major: [K-part, mh, M]
    w1x_in = nc.declare_dram_parameter("w1x", [128, 2, 128], b16, isOutput=False)
    w1a_in = nc.declare_dram_parameter("w1a", [128, 2, 128], b16, isOutput=False)
    w1u_in = nc.declare_dram_parameter("w1u", [64, 2, 128], b16, isOutput=False)
    w2_in = nc.declare_dram_parameter("w2", [128, 2, 128], b16, isOutput=False)
    b1_in = nc.declare_dram_parameter("b1", [128, 2], f32, isOutput=False)
    b2_in = nc.declare_dram_parameter("b2", [128, 1], f32, isOutput=False)
    outT = nc.declare_dram_parameter("outT", [128, npad], b16, isOutput=True)

    with tile.TileContext(nc) as tc:
        with (
            tc.tile_pool(name="const", bufs=1) as cpool,
            tc.tile_pool(name="xt", bufs=1) as xpool,
            tc.tile_pool(name="ea", bufs=ea_bufs) as eapool,
            tc.tile_pool(name="p", bufs=p_bufs) as ppool,
            tc.tile_pool(name="tail", bufs=max(1, 2 * len(tail_set))) as tailpool,
            tc.tile_pool(name="idx", bufs=3) as idxpool,
            tc.tile_pool(name="hag", bufs=3) as hagpool,
            tc.tile_pool(name="ug", bufs=2) as ugpool,
            tc.tile_pool(name="h1", bufs=4) as h1pool,
            tc.tile_pool(name="outs", bufs=2) as opool,
            tc.tile_pool(name="ps_agg", bufs=4, space="PSUM") as agg_ps_pool,
            tc.tile_pool(name="ps_o1", bufs=2, space="PSUM") as o1_ps_pool,
            tc.tile_pool(name="ps_o2", bufs=2, space="PSUM") as o2_ps_pool,
        ):
          def _emit_body():
              # ---- prefetch first groups' edge data (critical path) ----
              fetched = {}

              def fetch(g):
                  if g is None or g >= n_groups or g in fetched:
                      return
                  t0, t1 = gbounds[g]
                  gt = t1 - t0
                  if g in tail_set:
                      ea_t = tailpool.tile([128, gt * ECOL], fp8, tag="tail",
                                           name=f"tea{g}")
                      p_t = tailpool.tile([128, gt * 32], fp8, tag="tail",
                                          name=f"tp{g}")
                  else:
                      ea_t = eapool.tile([128, max_gt * ECOL], fp8, tag="ea",
                                         name=f"ea{g}")
                      p_t = ppool.tile([128, max_gt * 32], fp8, tag="p",
                                       name=f"p{g}")
                  nc.sync.dma_start(ea_t[:, : gt * ECOL],
                                    ea_in[:, t0 * ECOL : t1 * ECOL])
                  if is_devp(g):
                      idx_t = idxpool.tile([128, max_gt], b16, tag="idx",
                                           name=f"idx{g}")
                      nc.scalar.dma_start(idx_t[:, :gt], idx_in[:, t0:t1])
                      for jj, j in enumerate(range(0, gt, 8)):
                          r = min(8, gt - j)
                          out_ap = p_t[:, j * 32 : (j + r) * 32].rearrange(
                              "p (a m) -> p a m", a=r)
                          in1 = idx_t[:, j : j + r][:, :, None].broadcast_to(
                              [128, r, 32])
                          nc.vector.tensor_tensor(
                              out=out_ap, in0=iota_t[:, :r, :], in1=in1,
                              op=mybir.AluOpType.is_equal)
                  else:
                      nc.scalar.dma_start(p_t[:, : gt * 32],
                                          p_in[:, t0 * 32 : t1 * 32])
                  fetched[g] = (ea_t, p_t)

              iota_t = cpool.tile([128, 8, 32], b16, tag="iota")
              nc.gpsimd.dma_start(iota_t[:], iota_in[:])

              prefetch = min(opts.get("prefetch", 2), n_groups)
              for g in range(prefetch):
                  fetch(g)
              for g in sorted(tail_set):
                  fetch(g)

              # processing order: mids, then tails, then the last mid group
              porder = [g for g in range(n_groups) if g not in tail_set]
              porder = porder[:-1] + sorted(tail_set) + porder[-1:]

              # ---- constants on the gpsimd ring (off the critical path) ----
              w1x_t = cpool.tile([128, 2, 128], b16, tag="w1x")
              nc.gpsimd.dma_start(w1x_t[:], w1x_in[:])
              w1a_t = cpool.tile([128, 2, 128], b16, tag="w1a")
              nc.gpsimd.dma_start(w1a_t[:], w1a_in[:])
              w1u_t = cpool.tile([64, 2, 128], b16, tag="w1u")
              nc.gpsimd.dma_start(w1u_t[:], w1u_in[:])
              w2_t = cpool.tile([128, 2, 128], b16, tag="w2")
              nc.gpsimd.dma_start(w2_t[:], w2_in[:])
              b1_t = cpool.tile([128, 2], f32, tag="b1")
              nc.gpsimd.dma_start(b1_t[:], b1_in[:])
              b2_t = cpool.tile([128, 1], f32, tag="b2")
              nc.gpsimd.dma_start(b2_t[:], b2_in[:])

              xT_t = xpool.tile([128, npad], fp8, tag="xT")
              # load x in chunks so early groups can start sooner
              xchunk = 16 * NB
              for s in range(0, npad, xchunk):
                  e = min(s + xchunk, npad)
                  nc.sync.dma_start(xT_t[:, s:e], xT_in[:, s:e])

              for gi, g in enumerate(porder):
                  nb = gsizes[g]
                  gw = nb * NB
                  s = gstart[g] * NB
                  t0, t1 = gbounds[g]
                  fetch(g)
                  fetch(porder[gi + 1] if gi + 1 < len(porder) else None)
                  fetch(porder[gi + 2] if gi + 2 < len(porder) else None)
                  ea_t, p_t = fetched.pop(g)
                  ug_t = ugpool.tile([64, group * NB], fp8, tag="ug")
                  nc.scalar.dma_start(ug_t[:, :gw], ugT_in[:, s : s + gw])
                  hag = hagpool.tile([128, group * NB], b16, tag="hag",
                                     name=f"hag{g}")
                  dma_only = opts.get("dma_only")
                  for bi in range(nb if not dma_only else 0):
                      b = gstart[g] + bi
                      agg_ps = agg_ps_pool.tile([128, NB], f32, tag="agg")
                      for w in range(WIN):
                          wi = b * WIN + w
                          nt = 1 if opts.get("probe_no_edge_mm") else Tb[wi]
                          for t in range(nt):
                              ti = offs[wi] - t0 + t
                              aslice = (agg_ps[:, WSZ * w : WSZ * (w + 1)]
                                        if ECOL == 128 else
                                        agg_ps[0:64, WSZ * w : WSZ * (w + 1)])
                              nc.tensor.matmul(
                                  aslice,
                                  ea_t[:, ti * ECOL : ti * ECOL + ECOL],
                                  p_t[:, ti * 32 : (ti + 1) * 32],
                                  start=(t == 0),
                                  stop=(t == nt - 1),
                              )
                      # hi|lo partial sums -> MLP staging tile (DVE copy)
                      nc.vector.tensor_scalar(
                          out=hag[:, bi * NB : (bi + 1) * NB],
                          in0=agg_ps[:],
                          scalar1=0.0, scalar2=None,
                          op0=mybir.AluOpType.add,
                      )

                  # ---- MLP for this group (feature-major, bf16) ----
                  if dma_only:
                      out_t = opool.tile([128, gw], b16, tag="outs",
                                         name=f"dout{g}")
                      nc.gpsimd.dma_start(outT[:, s : s + gw], out_t[:, :gw])
                      continue
                  h1_list = []
                  for mh in range(2):
                      o1 = o1_ps_pool.tile([128, group * NB], f32, tag="o1")
                      nc.tensor.matmul(
                          o1[:, :gw], w1x_t[:, mh, :],
                          xT_t[:, s : s + gw],
                          start=True, stop=False,
                      )
                      nc.tensor.matmul(
                          o1[:, :gw], w1a_t[:, mh, :],
                          hag[:, :gw],
                          start=False, stop=False,
                      )
                      nc.tensor.matmul(
                          o1[:, :gw], w1u_t[:, mh, :],
                          ug_t[:, :gw],
                          start=False, stop=True,
                      )
                      h1 = h1pool.tile([128, group * NB], b16, tag="h1")
                      nc.scalar.activation(
                          out=h1[:, :gw], in_=o1[:, :gw],
                          func=mybir.ActivationFunctionType.Relu,
                          bias=b1_t[:, mh : mh + 1],
                      )
                      h1_list.append(h1)
                  o2 = o2_ps_pool.tile([128, group * NB], f32, tag="o2")
                  for kh in range(2):
                      nc.tensor.matmul(
                          o2[:, :gw], w2_t[:, kh, :],
                          h1_list[kh][:, :gw],
                          start=(kh == 0), stop=(kh == 1),
                      )
                  out_t = opool.tile([128, group * NB], b16, tag="outs")
                  nc.scalar.activation(
                      out=out_t[:, :gw], in_=o2[:, :gw],
                      func=mybir.ActivationFunctionType.Identity,
                      bias=b2_t[:],
                  )
                  nc.gpsimd.dma_start(outT[:, s : s + gw], out_t[:, :gw])

          if reps == 1:
              _emit_body()
          else:
              with tc.For_i(0, reps, 1):
                  _emit_body()

    nc.compile()
    return nc


def _node_permutation(row, n_nodes, n_cores, blocks):
    """Degree-balanced snake deal of nodes into 32-slot windows.

    Returns perm (slot -> node id, -1 for pad) with every window's edge
    count <= WSZ*16 = 512 (verified: max is exactly 512 for this input)."""
    nwin_total = n_cores * blocks * WIN
    deg = np.bincount(row, minlength=n_nodes)
    order = np.argsort(-deg, kind="stable")
    padded = np.full(WSZ * nwin_total, -1, np.int64)
    padded[:n_nodes] = order
    bands = padded.reshape(WSZ, nwin_total).copy()
    for r in range(1, WSZ, 2):
        bands[r] = bands[r][::-1]
    # slot = window*32 + band  ->  perm[slot] = bands[band, window]
    return bands.T.reshape(-1)  # [nwin_total * 32]


def _pack_inputs(x, edge_index, edge_attr, u, v_indices, W1, b1, W2, b2, cfg):
    """Host-side sharding: permute nodes, bucket+pack edges, build fp8 P."""
    n_cores, npc, blocks = cfg["n_cores"], cfg["npc"], cfg["blocks"]
    n_nodes = cfg["n_nodes"]
    npad = blocks * NB
    assert npc == npad
    row = np.asarray(edge_index[0], dtype=np.int64)
    ea = np.ascontiguousarray(np.asarray(edge_attr, dtype=np.float32))
    x = np.asarray(x, dtype=np.float32)
    u = np.asarray(u, dtype=np.float32)
    v_indices = np.asarray(v_indices, dtype=np.int64)
    W1 = np.asarray(W1, dtype=np.float32)
    W2 = np.asarray(W2, dtype=np.float32)
    b1 = np.asarray(b1, dtype=np.float32)
    b2 = np.asarray(b2, dtype=np.float32)
    d_e = ea.shape[1]
    nwin = blocks * WIN
    nwin_total = n_cores * nwin

    perm = _node_permutation(row, n_nodes, n_cores, blocks)
    slot_of_node = np.empty(n_nodes, np.int64)
    valid = perm >= 0
    slot_of_node[perm[valid]] = np.nonzero(valid)[0]

    eslot = slot_of_node[row]             # destination slot per edge
    ewin = eslot >> 5                     # global window id
    ecol = (eslot & 31).astype(np.int64)  # within-window one-hot column
    order_e = np.argsort(ewin, kind="stable")
    ewin_s = ewin[order_e]
    ecol_s = ecol[order_e]

    cnt = np.bincount(ewin_s, minlength=nwin_total)
    Tg = np.maximum(1, (cnt + 127) // 128)            # tiles per window
    Tb = Tg.reshape(n_cores, nwin).max(axis=0)        # shared across cores
    offs = np.concatenate([[0], np.cumsum(Tb)])
    TT = int(offs[-1])

    # fp8 hi/lo split of edge_attr (sorted by destination window)
    ea_s = ea[order_e]
    hi = ea_s.astype(f8)
    lo = (ea_s - hi.astype(np.float32)).astype(f8)
    ea_hilo = np.concatenate([hi, lo], axis=1)        # [E, 128] fp8

    wstart = np.concatenate([[0], np.cumsum(cnt)])    # edge range per window
    ea_pack = np.empty((n_cores, 128, TT * 128), dtype=f8)
    p_pack = np.empty((n_cores, 128, TT * 32), dtype=f8)
    idx_pack = np.empty((n_cores, 128, TT), dtype=bf16)
    for c in range(n_cores):
        cs, ce = wstart[c * nwin], wstart[(c + 1) * nwin]
        cnt_c = cnt[c * nwin : (c + 1) * nwin]
        wl = np.repeat(np.arange(nwin), cnt_c)        # local window per edge
        rank = np.arange(ce - cs) - np.repeat(wstart[c * nwin : (c + 1) * nwin] - cs, cnt_c)
        slot = offs[wl] * 128 + rank                  # tile-slot per edge
        coreslots = np.zeros((TT * 128, 2 * d_e), dtype=f8)
        coreslots[slot] = ea_hilo[cs:ce]
        ea_pack[c] = (
            coreslots.reshape(TT, 128, 2 * d_e)
            .transpose(1, 0, 2)
            .reshape(128, TT * 128)
        )
        pvals = np.zeros((TT * 128, 32), dtype=f8)
        pvals[slot, ecol_s[cs:ce]] = 1.0
        p_pack[c] = (
            pvals.reshape(TT, 128, 32).transpose(1, 0, 2).reshape(128, TT * 32)
        )
        ivals = np.full(TT * 128, -1.0, dtype=bf16)
        ivals[slot] = ecol_s[cs:ce]
        idx_pack[c] = ivals.reshape(TT, 128).T

    uT = u.T  # [d_u, n_graphs]
    perm_c = np.where(valid, perm, 0)
    iota4 = np.ascontiguousarray(
        np.broadcast_to(np.arange(32, dtype=np.float32), (128, 8, 32))
    ).astype(bf16)

    # weights, partition-major [K, mh, M], bf16
    w1x = np.ascontiguousarray(W1[:D_X].reshape(D_X, 2, 128)).astype(bf16)
    w1a_single = W1[D_X : D_X + d_e]                       # [64, 256]
    w1a_dup = np.concatenate([w1a_single, w1a_single], 0)  # [128, 256] hi|lo dup
    w1a = np.ascontiguousarray(w1a_dup.reshape(128, 2, 128)).astype(bf16)
    w1u = np.ascontiguousarray(W1[D_X + d_e :].reshape(D_U, 2, 128)).astype(bf16)
    w2 = np.ascontiguousarray(W2.reshape(2, 128, D_OUT).transpose(1, 0, 2)).astype(bf16)
    b1p = np.ascontiguousarray(b1.reshape(2, 128).T)
    b2p = np.ascontiguousarray(b2.reshape(128, 1))

    in_maps = []
    for c in range(n_cores):
        sl = slice(c * npad, (c + 1) * npad)
        xT = np.ascontiguousarray(x[perm_c[sl]].T).astype(f8)
        ugT = np.ascontiguousarray(uT[:, v_indices[perm_c[sl]]]).astype(f8)
        in_maps.append({
            "ea": ea_pack[c],
            "p": p_pack[c],
            "idx": idx_pack[c],
            "iota": iota4,
            "xT": xT,
            "ugT": ugT,
            "w1x": w1x,
            "w1a": w1a,
            "w1u": w1u,
            "w2": w2,
            "b1": b1p,
            "b2": b2p,
        })
    return in_maps, tuple(int(t) for t in Tb), perm


def _unscramble(outT_list, perm, cfg):
    """[core][128, npad] bf16 device outputs -> [n_nodes, D_OUT] f32."""
    n_nodes, npad = cfg["n_nodes"], cfg["blocks"] * NB
    out = np.empty((n_nodes, D_OUT), dtype=np.float32)
    for c in range(cfg["n_cores"]):
        pc = perm[c * npad : (c + 1) * npad]
        m = pc >= 0
        out[pc[m]] = np.asarray(outT_list[c]).T[m].astype(np.float32)
    return out


def _run(inputs, cfg, trace=False, reps=1):
    in_maps, T, perm = _pack_inputs(
        inputs["x"], inputs["edge_index"], inputs["edge_attr"], inputs["u"],
        inputs["v_indices"], inputs["W1"], inputs["b1"], inputs["W2"],
        inputs["b2"], cfg,
    )
    key = (T, cfg["blocks"], cfg["group"], reps)
    if key not in _cache:
        _cache[key] = _build_nc(
            T, cfg["blocks"], cfg["blocks"] * NB, cfg["group"], reps=reps
        )
    nc = _cache[key]
    res = run_bass_kernel_spmd(nc, in_maps, list(range(cfg["n_cores"])), trace=trace)
    out = _unscramble([res.results[c]["outT"] for c in range(cfg["n_cores"])],
                      perm, cfg)
    _run.last_results = res
    return out


def kernel(x, edge_index, edge_attr, u, v_indices, W1, b1, W2, b2):
    inputs = dict(x=x, edge_index=edge_index, edge_attr=edge_attr, u=u,
                  v_indices=v_indices, W1=W1, b1=b1, W2=W2, b2=b2)
    return _run(inputs, FULL_CFG)
